# revision 24
# baseline (speedup 1.0000x reference)
"""Trainium2 Bass kernel for grouped-expert 3-layer MLP (MoE, known covariance).

Computes, for x[B, E, DIN] and per-expert weights:
    h1 = relu(x[:,e] @ W1[e] + b1[e])      # [B, H]
    h2 = relu(h1 @ W2[e] + b2[e])          # [B, H]
    o  = h2 @ W3[e] + b3[e]                # [B, DOUT]
    out = sum_e o                          # [B, DOUT]

Sharding: data-parallel over batch across 8 NeuronCores (B=8192 -> 1024/core).
Weights are replicated to every core; no collectives needed.

Per-core schedule (v3):
  - All matmuls run in bf16 (1 cycle/row on the PE; fp8 would be 2x but its
    quantization noise measures ~3-6e-2 on this problem vs the 2e-2 gate).
    Inputs are cast fp32->bf16 in-flight by the gpsimd SWDGE DMA.
  - Expert-outer loop with double-buffered weights: the 23MB weight stream is
    spread over the whole kernel (~150GB/s); a block-outer variant needs 2x
    that and starves the PE (measured).
  - x tiles are PE-transposed to feature-major in bf16 (1 cyc/row). The
    transposes for expert e+1 are emitted between L1(e) and L2(e) so their
    PSUM->SBUF evacuation is complete long before L1(e+1) consumes them.
  - Layer 3 (M=DOUT=64) runs column-tiled: gb even -> PSUM partitions 0:64,
    gb odd -> 64:128, two concurrent matmuls in separate PE column groups,
    accumulated over all experts; the halves are summed in the epilogue.
  - L3 of expert e is emitted after L1 of expert e+1 (software stagger) so its
    h2-evacuation dependency is met without stalling the PE.
  - A short burst of identity matmuls at t=0 warms the PE HAM clock gate
    (cold PE runs at 1.2GHz for the first ~3.4us otherwise).
"""

import os
from contextlib import ExitStack

import bass_rust
import numpy as np

import concourse.bass as bass
import concourse.tile as tile
from concourse import bacc, mybir
from concourse.bass_utils import run_bass_kernel_spmd
from concourse.masks import make_identity

E, DIN, H, DOUT = 16, 128, 512, 64
B_FULL = 8192
N_CORES = 8
HB = H // 128  # 4 h-blocks
F32 = mybir.dt.float32
BF = mybir.dt.bfloat16


def build_nc(bloc=B_FULL // N_CORES, nb=512, n_warm=12):
    nbt = bloc // nb  # batch tiles per core
    nt = nb // 128
    assert bloc % nb == 0 and nb % 128 == 0

    nc = bacc.Bacc("TRN2", target_bir_lowering=False, debug=False)

    x = nc.dram_tensor("x", [bloc, E, DIN], F32, kind="ExternalInput")
    W1 = nc.dram_tensor("W1", [E, DIN, H], F32, kind="ExternalInput")
    b1 = nc.dram_tensor("b1", [E, H], F32, kind="ExternalInput")
    W2 = nc.dram_tensor("W2", [E, H, H], F32, kind="ExternalInput")
    b2 = nc.dram_tensor("b2", [E, H], F32, kind="ExternalInput")
    W3 = nc.dram_tensor("W3", [E, H, DOUT], F32, kind="ExternalInput")
    b3 = nc.dram_tensor("b3", [E, DOUT], F32, kind="ExternalInput")
    out = nc.dram_tensor("out", [bloc, DOUT], F32, kind="ExternalOutput")

    RELU = mybir.ActivationFunctionType.Relu
    ADD = mybir.AluOpType.add
    MAX = mybir.AluOpType.max

    with tile.TileContext(nc) as tc, ExitStack() as ctx:
        consts = ctx.enter_context(tc.tile_pool(name="consts", bufs=1))
        w1p = ctx.enter_context(tc.tile_pool(name="w1p", bufs=2))
        w2p = ctx.enter_context(tc.tile_pool(name="w2p", bufs=2))
        w3p = ctx.enter_context(tc.tile_pool(name="w3p", bufs=2))
        xp = ctx.enter_context(tc.tile_pool(name="xp", bufs=2))
        xtp = ctx.enter_context(tc.tile_pool(name="xtp", bufs=10))
        h1p = ctx.enter_context(tc.tile_pool(name="h1p", bufs=2))
        h2p = ctx.enter_context(tc.tile_pool(name="h2p", bufs=2))
        obp = ctx.enter_context(tc.tile_pool(name="obp", bufs=2))
        p1p = ctx.enter_context(tc.tile_pool(name="p1p", bufs=3, space="PSUM"))
        p2p = ctx.enter_context(tc.tile_pool(name="p2p", bufs=3, space="PSUM"))
        pop = ctx.enter_context(tc.tile_pool(name="pop", bufs=2, space="PSUM"))

        ident = consts.tile([128, 128], F32)
        make_identity(nc, ident)
        identr = consts.tile([128, 128], BF)
        nc.scalar.copy(identr, ident)

        # PE warmup: real (non-transpose) matmuls so the HAM clock gate sees
        # sustained activity and unthrottles 1.2 -> 2.4GHz before the first
        # data-dependent matmul issues.
        pjunk = p1p.tile([128, nb], F32, tag="p1", name="junk")
        jrhs = consts.tile([128, nb], BF)
        nc.vector.tensor_copy(jrhs[:, :128], identr)
        nc.vector.tensor_copy(jrhs[:, 128:256], identr)
        nc.vector.tensor_copy(jrhs[:, 256:384], identr)
        nc.vector.tensor_copy(jrhs[:, 384:], identr)
        for _ in range(n_warm):
            nc.tensor.matmul(pjunk, identr, jrhs, start=True, stop=True)

        # biases: load natural layout, PE-transpose so the per-feature bias
        # lands on partitions: b1s[p, hb*E + e] = b1[e, hb*128 + p]
        b1n = consts.tile([E, H], F32)
        nc.sync.dma_start(out=b1n, in_=b1[:, :])
        b2n = consts.tile([E, H], F32)
        nc.sync.dma_start(out=b2n, in_=b2[:, :])
        b3n = consts.tile([E, DOUT], F32)
        nc.sync.dma_start(out=b3n, in_=b3[:, :])
        b1s = consts.tile([128, HB * E], F32)
        b2s = consts.tile([128, HB * E], F32)
        for bn, bs in ((b1n, b1s), (b2n, b2s)):
            pb = p2p.tile([128, HB * E], F32, tag="p2", name="pb")
            for hb in range(HB):
                nc.tensor.transpose(
                    pb[:, hb * E : (hb + 1) * E],
                    bn[:, hb * 128 : (hb + 1) * 128],
                    ident[:E, :E],
                )
            nc.vector.tensor_copy(bs, pb)
        pb3 = p2p.tile([DOUT, E], F32, tag="p2", name="pb3")
        nc.tensor.transpose(pb3, b3n, ident[:E, :E])
        b3s = consts.tile([DOUT, E], F32)
        nc.vector.tensor_copy(b3s, pb3)
        b3sum = consts.tile([DOUT, 1], F32)
        nc.vector.reduce_sum(b3sum, b3s, axis=bass_rust.AxisListType.X)

        # relu+bias PSUM evacuation, rotated over 3 engines (ACT/DVE/GpSimd)
        # so consecutive PSUM-ring slots free up in parallel
        def evac_relu(out_ap, ps, bias, k):
            if k % 2 == 0:
                nc.scalar.activation(out_ap, ps, RELU, bias=bias)
            else:
                nc.vector.tensor_scalar(out_ap, ps, bias, 0.0, ADD, MAX)

        # ---- DMA emission helpers (gpsimd SWDGE, cast fp32->bf16) ----
        xin_tiles = {}

        def load_x(e):
            t = xp.tile([128, nbt, nt, DIN], BF, tag="xin", name="xin")
            nc.gpsimd.dma_start(
                out=t,
                in_=x[:, e, :].rearrange("(bt t p) d -> p bt t d", p=128, bt=nbt),
            )
            xin_tiles[e] = t

        w_tiles = {}

        def load_w(e0, ne):
            # one DMA instruction per weight tensor covering `ne` experts --
            # the ~1us fixed SWDGE issue cost dominates per-instruction
            es = slice(e0, e0 + ne)
            w1t = w1p.tile([DIN, ne, H], BF, tag="w1", name="w1")
            nc.gpsimd.dma_start(out=w1t, in_=W1[es].rearrange("e d h -> d e h"))
            w2t = w2p.tile([128, ne, HB, H], BF, tag="w2", name="w2")
            nc.gpsimd.dma_start(
                out=w2t, in_=W2[es].rearrange("e (hb p) g -> p e hb g", p=128)
            )
            w3t = w3p.tile([128, ne, HB, DOUT], BF, tag="w3", name="w3")
            nc.gpsimd.dma_start(
                out=w3t, in_=W3[es].rearrange("e (gb p) o -> p e gb o", p=128)
            )
            for i in range(ne):
                w_tiles[e0 + i] = (w1t, w2t, w3t, i)

        # bootstrap loads: x first so the first transposes aren't stuck
        # behind the big weight transfers; single-expert weight loads up
        # front so L1/L2 of expert 0 start as early as possible
        load_x(0)
        load_w(0, 1)
        load_x(1)
        load_w(1, 1)

        xt_tiles = {}

        def make_transpose_emitters(e):
            # PE transpose xin -> feature-major xt; one emitter per batch
            # tile so the transposes can be interleaved between L1 matmuls.
            # The PSUM staging tile comes from the p1 pool, claimed lazily at
            # first emitter call so it slots between the L1 allocations.
            xin = xin_tiles.pop(e)
            box = {}

            def mk(bt):
                def go():
                    if "pxt" not in box:
                        box["pxt"] = p2p.tile(
                            [DIN, nbt * nb], BF, tag="p2", name="pxt"
                        )
                    pxt = box["pxt"]
                    for t in range(nt):
                        nc.tensor.transpose(
                            pxt[:, bt * nb + t * 128 : bt * nb + (t + 1) * 128],
                            xin[:, bt, t, :],
                            identr,
                        )
                    xt = xtp.tile([DIN, nb], BF, tag="xt", name="xt")
                    if bt % 2 == 0:
                        nc.scalar.copy(xt, pxt[:, bt * nb : (bt + 1) * nb])
                    else:
                        nc.vector.tensor_copy(xt, pxt[:, bt * nb : (bt + 1) * nb])
                    xt_tiles[(e, bt)] = xt

                return go

            return [mk(bt) for bt in range(nbt)]

        for fn in make_transpose_emitters(0):
            fn()

        pending = []  # staggered L3 pair-emitters from the previous expert

        for e in range(E):
            if e + 2 < E:
                load_x(e + 2)
            if e % 2 == 0 and e + 2 < E:
                load_w(e + 2, 2)
            w1t, w2t, w3t, wi = w_tiles.pop(e)

            # ---- layer 1 interleaved with prev-expert L3 pairs: L1 fills a
            # full PSUM bank every 213ns if issued back-to-back, outrunning
            # the ~700ns evacuations through the 3-slot ring; the L3 fillers
            # (which target the po banks) pace it so the ring never blocks
            # the PE. ----
            h1 = [
                h1p.tile([128, HB, nb], BF, tag=f"h1_{bt}", name=f"h1_{bt}")
                for bt in range(nbt)
            ]

            def emit_l1(bt, hb):
                ps = p1p.tile([128, nb], F32, tag="p1", name="ps1")
                nc.tensor.matmul(
                    ps,
                    w1t[:, wi, hb * 128 : (hb + 1) * 128],
                    xt_l1[bt],
                    start=True,
                    stop=True,
                )
                bias = b1s[:, hb * E + e : hb * E + e + 1]
                evac_relu(h1[bt][:, hb, :], ps, bias, bt * HB + hb)

            xt_l1 = [xt_tiles.pop((e, bt)) for bt in range(nbt)]
            trs = make_transpose_emitters(e + 1) if e + 1 < E else []
            fillers = ([trs[0]] if trs else []) + pending[:1]
            fillers2 = ([trs[1]] if len(trs) > 1 else []) + pending[1:2]
            l2_fillers = pending[2:]
            emit_l1(0, 0)
            for hb in range(1, HB):
                if fillers:
                    fillers.pop(0)()
                emit_l1(0, hb)
            for hb in range(HB):
                if fillers2:
                    fillers2.pop(0)()
                emit_l1(1, hb)
            for fn in fillers + fillers2:
                fn()
            pending = []

            # ---- layer 2, both batch tiles ----
            h2 = [h2p.tile([128, HB, nb], BF, tag=f"h2_{bt}", name=f"h2_{bt}") for bt in range(nbt)]
            for bt in range(nbt):
                if l2_fillers:
                    l2_fillers.pop(0)()
                for gb in range(HB):
                    ps = p2p.tile([128, nb], F32, tag="p2")
                    for hb in range(HB):
                        nc.tensor.matmul(
                            ps,
                            w2t[:, wi, hb, gb * 128 : (gb + 1) * 128],
                            h1[bt][:, hb, :],
                            start=(hb == 0),
                            stop=(hb == HB - 1),
                        )
                    bias = b2s[:, gb * E + e : gb * E + e + 1]
                    evac_relu(h2[bt][:, gb, :], ps, bias, bt * HB + gb + 1)

            # ---- layer 3: column-tiled accumulation into po, staggered.
            # Each pair is two concurrent matmuls in separate PE column
            # groups (tile_position (0,0) and (0,64)). ----
            def mk_l3(bt, g2, e=e, h2=h2, w3t=w3t, wi=wi):
                def go():
                    for gb in (2 * g2, 2 * g2 + 1):
                        half = (gb % 2) * DOUT
                        # two interleaved accumulation groups share the
                        # bank (column halves); per-element has_written
                        # bits keep this correct on HW
                        nc.tensor.matmul(
                            po[bt][half : half + DOUT, :],
                            w3t[:, wi, gb, :],
                            h2[bt][:, gb, :],
                            start=(e == 0 and gb < 2),
                            stop=(e == E - 1 and gb >= 2),
                            skip_group_check=True,
                        )

                return go

            if e == 0:
                po = [
                    pop.tile([128, nb], F32, tag="po", name=f"po{bt}")
                    for bt in range(nbt)
                ]
            pending = [mk_l3(bt, g2) for bt in range(nbt) for g2 in range(2)]

        for fn in pending:
            fn()

        # ---- epilogue: sum halves + bias, transpose to batch-major, store ----
        for bt in range(nbt):
            b0 = bt * nb
            ob = obp.tile([DOUT, nb], F32, tag="ob")
            # ob = (po_lo + b3sum) + po_hi — two ops; the engine may read only
            # one PSUM operand per instruction
            nc.vector.tensor_scalar_add(ob, po[bt][0:DOUT, :], b3sum)
            nc.vector.tensor_add(ob, ob, po[bt][DOUT : 2 * DOUT, :])
            pot = pop.tile([128, nt * DOUT], F32, tag="po", name=f"pot{bt}")
            for t in range(nt):
                nc.tensor.transpose(
                    pot[:, t * DOUT : (t + 1) * DOUT],
                    ob[:, t * 128 : (t + 1) * 128],
                    ident[:DOUT, :DOUT],
                )
            obt = obp.tile([128, nt * DOUT], F32, tag="obt")
            nc.vector.tensor_copy(obt, pot)
            nc.sync.dma_start(
                out=out[b0 : b0 + nb, :].rearrange("(t p) o -> p t o", p=128),
                in_=obt.rearrange("p (t o) -> p t o", o=DOUT),
            )

    nc.compile()
    return nc


_NC_CACHE = {}


def _get_nc():
    n_warm = int(os.environ.get("KERNEL_WARM", "12"))
    if n_warm not in _NC_CACHE:
        _NC_CACHE[n_warm] = build_nc(n_warm=n_warm)
    return _NC_CACHE[n_warm]


def kernel(x, W1, b1, W2, b2, W3, b3):
    x = np.ascontiguousarray(np.asarray(x, dtype=np.float32))
    ws = {
        "W1": np.ascontiguousarray(np.asarray(W1, dtype=np.float32)),
        "b1": np.ascontiguousarray(np.asarray(b1, dtype=np.float32)),
        "W2": np.ascontiguousarray(np.asarray(W2, dtype=np.float32)),
        "b2": np.ascontiguousarray(np.asarray(b2, dtype=np.float32)),
        "W3": np.ascontiguousarray(np.asarray(W3, dtype=np.float32)),
        "b3": np.ascontiguousarray(np.asarray(b3, dtype=np.float32)),
    }
    nc = _get_nc()
    shards = np.split(x, N_CORES, axis=0)
    in_maps = [{"x": np.ascontiguousarray(s), **ws} for s in shards]
    trace = bool(int(os.environ.get("KERNEL_TRACE", "0")))
    kwargs = {}
    if trace and os.environ.get("KERNEL_TRACE_DIR"):
        kwargs["tmpdir"] = os.environ["KERNEL_TRACE_DIR"]
    res = run_bass_kernel_spmd(nc, in_maps, list(range(N_CORES)), trace=trace, **kwargs)
    if trace:
        kernel.last_results = res
    return np.concatenate([res.results[c]["out"] for c in range(N_CORES)], axis=0)


# revision 25
# speedup vs baseline: 1.0287x; 1.0287x over previous
"""Trainium2 Bass kernel for grouped-expert 3-layer MLP (MoE, known covariance).

Computes, for x[B, E, DIN] and per-expert weights:
    h1 = relu(x[:,e] @ W1[e] + b1[e])      # [B, H]
    h2 = relu(h1 @ W2[e] + b2[e])          # [B, H]
    o  = h2 @ W3[e] + b3[e]                # [B, DOUT]
    out = sum_e o                          # [B, DOUT]

Sharding: data-parallel over batch across 8 NeuronCores (B=8192 -> 1024/core).
Weights are replicated to every core; no collectives needed.

Per-core schedule (v3):
  - All matmuls run in bf16 (1 cycle/row on the PE; fp8 would be 2x but its
    quantization noise measures ~3-6e-2 on this problem vs the 2e-2 gate).
    Inputs are cast fp32->bf16 in-flight by the gpsimd SWDGE DMA.
  - Expert-outer loop with double-buffered weights: the 23MB weight stream is
    spread over the whole kernel (~150GB/s); a block-outer variant needs 2x
    that and starves the PE (measured).
  - x tiles are PE-transposed to feature-major in bf16 (1 cyc/row). The
    transposes for expert e+1 are emitted between L1(e) and L2(e) so their
    PSUM->SBUF evacuation is complete long before L1(e+1) consumes them.
  - Layer 3 (M=DOUT=64) runs column-tiled: gb even -> PSUM partitions 0:64,
    gb odd -> 64:128, two concurrent matmuls in separate PE column groups,
    accumulated over all experts; the halves are summed in the epilogue.
  - L3 of expert e is emitted after L1 of expert e+1 (software stagger) so its
    h2-evacuation dependency is met without stalling the PE.
  - A short burst of identity matmuls at t=0 warms the PE HAM clock gate
    (cold PE runs at 1.2GHz for the first ~3.4us otherwise).
"""

import os
from contextlib import ExitStack

import bass_rust
import numpy as np

import concourse.bass as bass
import concourse.tile as tile
from concourse import bacc, mybir
from concourse.bass_utils import run_bass_kernel_spmd
from concourse.masks import make_identity

E, DIN, H, DOUT = 16, 128, 512, 64
B_FULL = 8192
N_CORES = 8
HB = H // 128  # 4 h-blocks
F32 = mybir.dt.float32
BF = mybir.dt.bfloat16


def build_nc(bloc=B_FULL // N_CORES, nb=512, n_warm=12):
    nbt = bloc // nb  # batch tiles per core
    nt = nb // 128
    assert bloc % nb == 0 and nb % 128 == 0

    nc = bacc.Bacc("TRN2", target_bir_lowering=False, debug=False)

    x = nc.dram_tensor("x", [bloc, E, DIN], F32, kind="ExternalInput")
    W1 = nc.dram_tensor("W1", [E, DIN, H], F32, kind="ExternalInput")
    b1 = nc.dram_tensor("b1", [E, H], F32, kind="ExternalInput")
    W2 = nc.dram_tensor("W2", [E, H, H], F32, kind="ExternalInput")
    b2 = nc.dram_tensor("b2", [E, H], F32, kind="ExternalInput")
    W3 = nc.dram_tensor("W3", [E, H, DOUT], F32, kind="ExternalInput")
    b3 = nc.dram_tensor("b3", [E, DOUT], F32, kind="ExternalInput")
    out = nc.dram_tensor("out", [bloc, DOUT], F32, kind="ExternalOutput")

    RELU = mybir.ActivationFunctionType.Relu
    ADD = mybir.AluOpType.add
    MAX = mybir.AluOpType.max

    with tile.TileContext(nc) as tc, ExitStack() as ctx:
        consts = ctx.enter_context(tc.tile_pool(name="consts", bufs=1))
        w1p = ctx.enter_context(tc.tile_pool(name="w1p", bufs=2))
        w2p = ctx.enter_context(tc.tile_pool(name="w2p", bufs=2))
        w3p = ctx.enter_context(tc.tile_pool(name="w3p", bufs=2))
        xp = ctx.enter_context(tc.tile_pool(name="xp", bufs=2))
        xtp = ctx.enter_context(tc.tile_pool(name="xtp", bufs=10))
        h1p = ctx.enter_context(tc.tile_pool(name="h1p", bufs=2))
        h2p = ctx.enter_context(tc.tile_pool(name="h2p", bufs=2))
        obp = ctx.enter_context(tc.tile_pool(name="obp", bufs=2))
        p1p = ctx.enter_context(tc.tile_pool(name="p1p", bufs=3, space="PSUM"))
        p2p = ctx.enter_context(tc.tile_pool(name="p2p", bufs=3, space="PSUM"))
        pop = ctx.enter_context(tc.tile_pool(name="pop", bufs=2, space="PSUM"))

        ident = consts.tile([128, 128], F32)
        make_identity(nc, ident)
        identr = consts.tile([128, 128], BF)
        nc.scalar.copy(identr, ident)

        # PE warmup: real (non-transpose) matmuls so the HAM clock gate sees
        # sustained activity and unthrottles 1.2 -> 2.4GHz before the first
        # data-dependent matmul issues.
        pjunk = p1p.tile([128, nb], F32, tag="p1", name="junk")
        jrhs = consts.tile([128, nb], BF)
        nc.vector.tensor_copy(jrhs[:, :128], identr)
        nc.vector.tensor_copy(jrhs[:, 128:256], identr)
        nc.vector.tensor_copy(jrhs[:, 256:384], identr)
        nc.vector.tensor_copy(jrhs[:, 384:], identr)
        for _ in range(n_warm):
            nc.tensor.matmul(pjunk, identr, jrhs, start=True, stop=True)

        # biases: load natural layout, PE-transpose so the per-feature bias
        # lands on partitions: b1s[p, hb*E + e] = b1[e, hb*128 + p]
        b1n = consts.tile([E, H], F32)
        nc.sync.dma_start(out=b1n, in_=b1[:, :])
        b2n = consts.tile([E, H], F32)
        nc.sync.dma_start(out=b2n, in_=b2[:, :])
        b3n = consts.tile([E, DOUT], F32)
        nc.sync.dma_start(out=b3n, in_=b3[:, :])
        b1s = consts.tile([128, HB * E], F32)
        b2s = consts.tile([128, HB * E], F32)
        for bn, bs in ((b1n, b1s), (b2n, b2s)):
            pb = p2p.tile([128, HB * E], F32, tag="p2", name="pb")
            for hb in range(HB):
                nc.tensor.transpose(
                    pb[:, hb * E : (hb + 1) * E],
                    bn[:, hb * 128 : (hb + 1) * 128],
                    ident[:E, :E],
                )
            nc.vector.tensor_copy(bs, pb)
        pb3 = p2p.tile([DOUT, E], F32, tag="p2", name="pb3")
        nc.tensor.transpose(pb3, b3n, ident[:E, :E])
        b3s = consts.tile([DOUT, E], F32)
        nc.vector.tensor_copy(b3s, pb3)
        b3sum = consts.tile([DOUT, 1], F32)
        nc.vector.reduce_sum(b3sum, b3s, axis=bass_rust.AxisListType.X)

        # relu+bias PSUM evacuation, rotated over 3 engines (ACT/DVE/GpSimd)
        # so consecutive PSUM-ring slots free up in parallel
        def evac_relu(out_ap, ps, bias, k):
            if k % 2 == 0:
                nc.scalar.activation(out_ap, ps, RELU, bias=bias)
            else:
                nc.vector.tensor_scalar(out_ap, ps, bias, 0.0, ADD, MAX)

        # ---- DMA emission helpers (gpsimd SWDGE, cast fp32->bf16) ----
        xin_tiles = {}

        def load_x(e):
            t = xp.tile([128, nbt, nt, DIN], BF, tag="xin", name="xin")
            nc.gpsimd.dma_start(
                out=t,
                in_=x[:, e, :].rearrange("(bt t p) d -> p bt t d", p=128, bt=nbt),
            )
            xin_tiles[e] = t

        w_tiles = {}

        def load_w(e0, ne):
            # one DMA instruction per weight tensor covering `ne` experts --
            # the ~1us fixed SWDGE issue cost dominates per-instruction
            es = slice(e0, e0 + ne)
            w1t = w1p.tile([DIN, ne, H], BF, tag="w1", name="w1")
            nc.gpsimd.dma_start(out=w1t, in_=W1[es].rearrange("e d h -> d e h"))
            w2t = w2p.tile([128, ne, HB, H], BF, tag="w2", name="w2")
            nc.gpsimd.dma_start(
                out=w2t, in_=W2[es].rearrange("e (hb p) g -> p e hb g", p=128)
            )
            w3t = w3p.tile([128, ne, HB, DOUT], BF, tag="w3", name="w3")
            nc.gpsimd.dma_start(
                out=w3t, in_=W3[es].rearrange("e (gb p) o -> p e gb o", p=128)
            )
            for i in range(ne):
                w_tiles[e0 + i] = (w1t, w2t, w3t, i)

        # bootstrap loads: x first so the first transposes aren't stuck
        # behind the big weight transfers; single-expert weight loads up
        # front so L1/L2 of expert 0 start as early as possible
        load_x(0)
        load_w(0, 1)
        load_x(1)
        load_w(1, 1)

        xt_tiles = {}

        def make_transpose_emitters(e):
            # PE transpose xin -> feature-major xt; one emitter per batch
            # tile so the transposes can be interleaved between L1 matmuls.
            # The PSUM staging tile comes from the p1 pool, claimed lazily at
            # first emitter call so it slots between the L1 allocations.
            xin = xin_tiles.pop(e)
            box = {}

            def mk(bt):
                def go():
                    if "pxt" not in box:
                        box["pxt"] = p2p.tile(
                            [DIN, nbt * nb], BF, tag="p2", name="pxt"
                        )
                    pxt = box["pxt"]
                    for t in range(nt):
                        nc.tensor.transpose(
                            pxt[:, bt * nb + t * 128 : bt * nb + (t + 1) * 128],
                            xin[:, bt, t, :],
                            identr,
                        )
                    xt = xtp.tile([DIN, nb], BF, tag="xt", name="xt")
                    if bt % 2 == 0:
                        nc.scalar.copy(xt, pxt[:, bt * nb : (bt + 1) * nb])
                    else:
                        nc.vector.tensor_copy(xt, pxt[:, bt * nb : (bt + 1) * nb])
                    xt_tiles[(e, bt)] = xt

                return go

            return [mk(bt) for bt in range(nbt)]

        for fn in make_transpose_emitters(0):
            fn()

        pending = []  # staggered L3 pair-emitters from the previous expert

        for e in range(E):
            if e + 2 < E:
                load_x(e + 2)
            if e % 2 == 0 and e + 2 < E:
                load_w(e + 2, 2)
            w1t, w2t, w3t, wi = w_tiles.pop(e)

            # ---- layer 1 interleaved with prev-expert L3 pairs: L1 fills a
            # full PSUM bank every 213ns if issued back-to-back, outrunning
            # the ~700ns evacuations through the 3-slot ring; the L3 fillers
            # (which target the po banks) pace it so the ring never blocks
            # the PE. ----
            h1 = [
                h1p.tile([128, HB, nb], BF, tag=f"h1_{bt}", name=f"h1_{bt}")
                for bt in range(nbt)
            ]

            def emit_l1(bt, hb):
                ps = p1p.tile([128, nb], F32, tag="p1", name="ps1")
                nc.tensor.matmul(
                    ps,
                    w1t[:, wi, hb * 128 : (hb + 1) * 128],
                    xt_l1[bt],
                    start=True,
                    stop=True,
                )
                bias = b1s[:, hb * E + e : hb * E + e + 1]
                evac_relu(h1[bt][:, hb, :], ps, bias, bt * HB + hb)

            xt_l1 = [xt_tiles.pop((e, bt)) for bt in range(nbt)]
            trs = make_transpose_emitters(e + 1) if e + 1 < E else []
            fillers = ([trs[0]] if trs else []) + pending[:2]
            fillers2 = ([trs[1]] if len(trs) > 1 else []) + pending[2:]
            emit_l1(0, 0)
            for hb in range(1, HB):
                if fillers:
                    fillers.pop(0)()
                emit_l1(0, hb)
            for hb in range(HB):
                if fillers2:
                    fillers2.pop(0)()
                emit_l1(1, hb)
            for fn in fillers + fillers2:
                fn()
            pending = []

            # ---- layer 2, both batch tiles ----
            h2 = [h2p.tile([128, HB, nb], BF, tag=f"h2_{bt}", name=f"h2_{bt}") for bt in range(nbt)]
            for bt in range(nbt):
                for gb in range(HB):
                    ps = p2p.tile([128, nb], F32, tag="p2")
                    for hb in range(HB):
                        nc.tensor.matmul(
                            ps,
                            w2t[:, wi, hb, gb * 128 : (gb + 1) * 128],
                            h1[bt][:, hb, :],
                            start=(hb == 0),
                            stop=(hb == HB - 1),
                        )
                    bias = b2s[:, gb * E + e : gb * E + e + 1]
                    evac_relu(h2[bt][:, gb, :], ps, bias, bt * HB + gb + 1)

            # ---- layer 3: column-tiled accumulation into po, staggered.
            # Each pair is two concurrent matmuls in separate PE column
            # groups (tile_position (0,0) and (0,64)). ----
            def mk_l3(bt, g2, e=e, h2=h2, w3t=w3t, wi=wi):
                def go():
                    for gb in (2 * g2, 2 * g2 + 1):
                        half = (gb % 2) * DOUT
                        # two interleaved accumulation groups share the
                        # bank (column halves); per-element has_written
                        # bits keep this correct on HW
                        nc.tensor.matmul(
                            po[bt][half : half + DOUT, :],
                            w3t[:, wi, gb, :],
                            h2[bt][:, gb, :],
                            start=(e == 0 and gb < 2),
                            stop=(e == E - 1 and gb >= 2),
                            skip_group_check=True,
                        )

                return go

            if e == 0:
                po = [
                    pop.tile([128, nb], F32, tag="po", name=f"po{bt}")
                    for bt in range(nbt)
                ]
            pending = [mk_l3(bt, g2) for bt in range(nbt) for g2 in range(2)]

        for fn in pending:
            fn()

        # ---- epilogue: sum halves + bias, transpose to batch-major, store ----
        for bt in range(nbt):
            b0 = bt * nb
            ob = obp.tile([DOUT, nb], F32, tag="ob")
            # ob = (po_lo + b3sum) + po_hi — two ops; the engine may read only
            # one PSUM operand per instruction
            nc.vector.tensor_scalar_add(ob, po[bt][0:DOUT, :], b3sum)
            nc.vector.tensor_add(ob, ob, po[bt][DOUT : 2 * DOUT, :])
            pot = pop.tile([128, nt * DOUT], F32, tag="po", name=f"pot{bt}")
            for t in range(nt):
                nc.tensor.transpose(
                    pot[:, t * DOUT : (t + 1) * DOUT],
                    ob[:, t * 128 : (t + 1) * 128],
                    ident[:DOUT, :DOUT],
                )
            obt = obp.tile([128, nt * DOUT], F32, tag="obt")
            nc.vector.tensor_copy(obt, pot)
            nc.sync.dma_start(
                out=out[b0 : b0 + nb, :].rearrange("(t p) o -> p t o", p=128),
                in_=obt.rearrange("p (t o) -> p t o", o=DOUT),
            )

    nc.compile()
    return nc


_NC_CACHE = {}


def _get_nc():
    n_warm = int(os.environ.get("KERNEL_WARM", "12"))
    if n_warm not in _NC_CACHE:
        _NC_CACHE[n_warm] = build_nc(n_warm=n_warm)
    return _NC_CACHE[n_warm]


def kernel(x, W1, b1, W2, b2, W3, b3):
    x = np.ascontiguousarray(np.asarray(x, dtype=np.float32))
    ws = {
        "W1": np.ascontiguousarray(np.asarray(W1, dtype=np.float32)),
        "b1": np.ascontiguousarray(np.asarray(b1, dtype=np.float32)),
        "W2": np.ascontiguousarray(np.asarray(W2, dtype=np.float32)),
        "b2": np.ascontiguousarray(np.asarray(b2, dtype=np.float32)),
        "W3": np.ascontiguousarray(np.asarray(W3, dtype=np.float32)),
        "b3": np.ascontiguousarray(np.asarray(b3, dtype=np.float32)),
    }
    nc = _get_nc()
    shards = np.split(x, N_CORES, axis=0)
    in_maps = [{"x": np.ascontiguousarray(s), **ws} for s in shards]
    trace = bool(int(os.environ.get("KERNEL_TRACE", "0")))
    kwargs = {}
    if trace and os.environ.get("KERNEL_TRACE_DIR"):
        kwargs["tmpdir"] = os.environ["KERNEL_TRACE_DIR"]
    res = run_bass_kernel_spmd(nc, in_maps, list(range(N_CORES)), trace=trace, **kwargs)
    if trace:
        kernel.last_results = res
    return np.concatenate([res.results[c]["out"] for c in range(N_CORES)], axis=0)


# revision 26
# speedup vs baseline: 1.0312x; 1.0025x over previous
"""Trainium2 Bass kernel for grouped-expert 3-layer MLP (MoE, known covariance).

Computes, for x[B, E, DIN] and per-expert weights:
    h1 = relu(x[:,e] @ W1[e] + b1[e])      # [B, H]
    h2 = relu(h1 @ W2[e] + b2[e])          # [B, H]
    o  = h2 @ W3[e] + b3[e]                # [B, DOUT]
    out = sum_e o                          # [B, DOUT]

Sharding: data-parallel over batch across 8 NeuronCores (B=8192 -> 1024/core).
Weights are replicated to every core; no collectives needed.

Per-core schedule:
  - All matmuls run in bf16 (1 cycle/row on the PE at N=512; fp8 DoubleRow
    would be ~1.4x more but its quantization noise measures 3-6e-2 on this
    problem vs the 2e-2 gate; bf16 lands at ~4e-3). Weights and x are cast
    fp32->bf16 in flight by the gpsimd SWDGE DMA.
  - Expert-outer loop with double-buffered weights: the 23MB weight stream
    spreads over the whole kernel. (A block-outer variant that keeps all
    weights resident needs 2x the DMA rate and starves the PE; an XBAR
    DMA-transpose path for x double-hops through DRAM and loses to DMA
    queue serialization -- both measured slower.)
  - Weight loads are batched two experts per DMA instruction: the ~1us
    fixed SWDGE issue cost dominates per-instruction.
  - x tiles are PE-transposed to feature-major in bf16 (1 cyc/row + FWL
    weight load, ~113ns per 128x128 tile). The transposes for expert e+1
    and the column-tiled L3 pairs of expert e-1 are interleaved between
    L1(e)'s matmuls: L1 fills a full PSUM bank every 213ns if issued
    back-to-back, outrunning the ~700ns evacuations through the 3-slot
    p1 ring; the fillers pace it so the ring never blocks the PE.
  - Layer 3 (M=DOUT=64) is column-tiled: gb even -> PSUM partitions 0:64,
    gb odd -> 64:128 of the same bank, two concurrent matmuls in separate
    PE column groups, accumulated across all 16 experts; the halves are
    summed in the epilogue.
  - PSUM budget (8 banks): p1=3 (L1), p2=3 (L2 + transpose staging), po=2
    (per-block expert-sum accumulators).
  - PSUM evacuations (relu+bias, fp32->bf16) alternate between the ACT and
    DVE engines (GpSimd cannot read PSUM on TRN2).
  - A ~5us burst of identity matmuls at t=0 warms the PE HAM clock gate
    during the initial DMA wait (a cold PE runs at 1.2GHz, and the gate
    needs a full 3.4us activity window to open).
"""

import os
from contextlib import ExitStack

import bass_rust
import numpy as np

import concourse.bass as bass
import concourse.tile as tile
from concourse import bacc, mybir
from concourse.bass_utils import run_bass_kernel_spmd
from concourse.masks import make_identity

E, DIN, H, DOUT = 16, 128, 512, 64
B_FULL = 8192
N_CORES = 8
HB = H // 128  # 4 h-blocks
F32 = mybir.dt.float32
BF = mybir.dt.bfloat16


def build_nc(bloc=B_FULL // N_CORES, nb=512, n_warm=12):
    nbt = bloc // nb  # batch tiles per core
    nt = nb // 128
    assert bloc % nb == 0 and nb % 128 == 0

    nc = bacc.Bacc("TRN2", target_bir_lowering=False, debug=False)

    x = nc.dram_tensor("x", [bloc, E, DIN], F32, kind="ExternalInput")
    W1 = nc.dram_tensor("W1", [E, DIN, H], F32, kind="ExternalInput")
    b1 = nc.dram_tensor("b1", [E, H], F32, kind="ExternalInput")
    W2 = nc.dram_tensor("W2", [E, H, H], F32, kind="ExternalInput")
    b2 = nc.dram_tensor("b2", [E, H], F32, kind="ExternalInput")
    W3 = nc.dram_tensor("W3", [E, H, DOUT], F32, kind="ExternalInput")
    b3 = nc.dram_tensor("b3", [E, DOUT], F32, kind="ExternalInput")
    out = nc.dram_tensor("out", [bloc, DOUT], F32, kind="ExternalOutput")

    RELU = mybir.ActivationFunctionType.Relu
    ADD = mybir.AluOpType.add
    MAX = mybir.AluOpType.max

    with tile.TileContext(nc) as tc, ExitStack() as ctx:
        consts = ctx.enter_context(tc.tile_pool(name="consts", bufs=1))
        w1p = ctx.enter_context(tc.tile_pool(name="w1p", bufs=2))
        w2p = ctx.enter_context(tc.tile_pool(name="w2p", bufs=2))
        w3p = ctx.enter_context(tc.tile_pool(name="w3p", bufs=2))
        xp = ctx.enter_context(tc.tile_pool(name="xp", bufs=2))
        xtp = ctx.enter_context(tc.tile_pool(name="xtp", bufs=10))
        h1p = ctx.enter_context(tc.tile_pool(name="h1p", bufs=2))
        h2p = ctx.enter_context(tc.tile_pool(name="h2p", bufs=2))
        obp = ctx.enter_context(tc.tile_pool(name="obp", bufs=2))
        p1p = ctx.enter_context(tc.tile_pool(name="p1p", bufs=3, space="PSUM"))
        p2p = ctx.enter_context(tc.tile_pool(name="p2p", bufs=3, space="PSUM"))
        pop = ctx.enter_context(tc.tile_pool(name="pop", bufs=2, space="PSUM"))

        ident = consts.tile([128, 128], F32)
        make_identity(nc, ident)
        identr = consts.tile([128, 128], BF)
        nc.scalar.copy(identr, ident)

        # PE warmup: real (non-transpose) matmuls so the HAM clock gate sees
        # sustained activity and unthrottles 1.2 -> 2.4GHz before the first
        # data-dependent matmul issues.
        pjunk = p1p.tile([128, nb], F32, tag="p1", name="junk")
        jrhs = consts.tile([128, nb], BF)
        nc.vector.tensor_copy(jrhs[:, :128], identr)
        nc.vector.tensor_copy(jrhs[:, 128:256], identr)
        nc.vector.tensor_copy(jrhs[:, 256:384], identr)
        nc.vector.tensor_copy(jrhs[:, 384:], identr)
        for _ in range(n_warm):
            nc.tensor.matmul(pjunk, identr, jrhs, start=True, stop=True)

        # biases: load natural layout, PE-transpose so the per-feature bias
        # lands on partitions: b1s[p, hb*E + e] = b1[e, hb*128 + p]
        b1n = consts.tile([E, H], F32)
        nc.sync.dma_start(out=b1n, in_=b1[:, :])
        b2n = consts.tile([E, H], F32)
        nc.sync.dma_start(out=b2n, in_=b2[:, :])
        b3n = consts.tile([E, DOUT], F32)
        nc.sync.dma_start(out=b3n, in_=b3[:, :])
        b1s = consts.tile([128, HB * E], F32)
        b2s = consts.tile([128, HB * E], F32)
        for bn, bs in ((b1n, b1s), (b2n, b2s)):
            pb = p2p.tile([128, HB * E], F32, tag="p2", name="pb")
            for hb in range(HB):
                nc.tensor.transpose(
                    pb[:, hb * E : (hb + 1) * E],
                    bn[:, hb * 128 : (hb + 1) * 128],
                    ident[:E, :E],
                )
            nc.vector.tensor_copy(bs, pb)
        pb3 = p2p.tile([DOUT, E], F32, tag="p2", name="pb3")
        nc.tensor.transpose(pb3, b3n, ident[:E, :E])
        b3s = consts.tile([DOUT, E], F32)
        nc.vector.tensor_copy(b3s, pb3)
        b3sum = consts.tile([DOUT, 1], F32)
        nc.vector.reduce_sum(b3sum, b3s, axis=bass_rust.AxisListType.X)

        # relu+bias PSUM evacuation, rotated over 3 engines (ACT/DVE/GpSimd)
        # so consecutive PSUM-ring slots free up in parallel
        def evac_relu(out_ap, ps, bias, k):
            if k % 2 == 0:
                nc.scalar.activation(out_ap, ps, RELU, bias=bias)
            else:
                nc.vector.tensor_scalar(out_ap, ps, bias, 0.0, ADD, MAX)

        # ---- DMA emission helpers (gpsimd SWDGE, cast fp32->bf16) ----
        xin_tiles = {}

        def load_x(e):
            t = xp.tile([128, nbt, nt, DIN], BF, tag="xin", name="xin")
            nc.gpsimd.dma_start(
                out=t,
                in_=x[:, e, :].rearrange("(bt t p) d -> p bt t d", p=128, bt=nbt),
            )
            xin_tiles[e] = t

        w_tiles = {}

        def load_w(e0, ne):
            # one DMA instruction per weight tensor covering `ne` experts --
            # the ~1us fixed SWDGE issue cost dominates per-instruction
            es = slice(e0, e0 + ne)
            w1t = w1p.tile([DIN, ne, H], BF, tag="w1", name="w1")
            nc.gpsimd.dma_start(out=w1t, in_=W1[es].rearrange("e d h -> d e h"))
            w2t = w2p.tile([128, ne, HB, H], BF, tag="w2", name="w2")
            nc.gpsimd.dma_start(
                out=w2t, in_=W2[es].rearrange("e (hb p) g -> p e hb g", p=128)
            )
            w3t = w3p.tile([128, ne, HB, DOUT], BF, tag="w3", name="w3")
            nc.gpsimd.dma_start(
                out=w3t, in_=W3[es].rearrange("e (gb p) o -> p e gb o", p=128)
            )
            for i in range(ne):
                w_tiles[e0 + i] = (w1t, w2t, w3t, i)

        # bootstrap loads: x first so the first transposes aren't stuck
        # behind the big weight transfers; single-expert weight loads up
        # front so L1/L2 of expert 0 start as early as possible
        load_x(0)
        load_w(0, 1)
        load_x(1)
        load_w(1, 1)

        xt_tiles = {}

        def make_transpose_emitters(e):
            # PE transpose xin -> feature-major xt; one emitter per batch
            # tile so the transposes can be interleaved between L1 matmuls.
            # The PSUM staging tile comes from the p1 pool, claimed lazily at
            # first emitter call so it slots between the L1 allocations.
            xin = xin_tiles.pop(e)
            box = {}

            def mk(bt):
                def go():
                    if "pxt" not in box:
                        box["pxt"] = p2p.tile(
                            [DIN, nbt * nb], BF, tag="p2", name="pxt"
                        )
                    pxt = box["pxt"]
                    for t in range(nt):
                        nc.tensor.transpose(
                            pxt[:, bt * nb + t * 128 : bt * nb + (t + 1) * 128],
                            xin[:, bt, t, :],
                            identr,
                        )
                    xt = xtp.tile([DIN, nb], BF, tag="xt", name="xt")
                    if bt % 2 == 0:
                        nc.scalar.copy(xt, pxt[:, bt * nb : (bt + 1) * nb])
                    else:
                        nc.vector.tensor_copy(xt, pxt[:, bt * nb : (bt + 1) * nb])
                    xt_tiles[(e, bt)] = xt

                return go

            return [mk(bt) for bt in range(nbt)]

        for fn in make_transpose_emitters(0):
            fn()

        pending = []  # staggered L3 pair-emitters from the previous expert

        for e in range(E):
            if e + 2 < E:
                load_x(e + 2)
            if e % 2 == 0 and e + 2 < E:
                load_w(e + 2, 2)
            w1t, w2t, w3t, wi = w_tiles.pop(e)

            # ---- layer 1 interleaved with prev-expert L3 pairs: L1 fills a
            # full PSUM bank every 213ns if issued back-to-back, outrunning
            # the ~700ns evacuations through the 3-slot ring; the L3 fillers
            # (which target the po banks) pace it so the ring never blocks
            # the PE. ----
            h1 = [
                h1p.tile([128, HB, nb], BF, tag=f"h1_{bt}", name=f"h1_{bt}")
                for bt in range(nbt)
            ]

            def emit_l1(bt, hb):
                ps = p1p.tile([128, nb], F32, tag="p1", name="ps1")
                nc.tensor.matmul(
                    ps,
                    w1t[:, wi, hb * 128 : (hb + 1) * 128],
                    xt_l1[bt],
                    start=True,
                    stop=True,
                )
                bias = b1s[:, hb * E + e : hb * E + e + 1]
                evac_relu(h1[bt][:, hb, :], ps, bias, bt * HB + hb)

            xt_l1 = [xt_tiles.pop((e, bt)) for bt in range(nbt)]
            trs = make_transpose_emitters(e + 1) if e + 1 < E else []
            fillers = ([trs[0]] if trs else []) + pending[:2]
            fillers2 = ([trs[1]] if len(trs) > 1 else []) + pending[2:]
            emit_l1(0, 0)
            for hb in range(1, HB):
                if fillers:
                    fillers.pop(0)()
                emit_l1(0, hb)
            for hb in range(HB):
                if fillers2:
                    fillers2.pop(0)()
                emit_l1(1, hb)
            for fn in fillers + fillers2:
                fn()
            pending = []

            # ---- layer 2, both batch tiles ----
            h2 = [h2p.tile([128, HB, nb], BF, tag=f"h2_{bt}", name=f"h2_{bt}") for bt in range(nbt)]
            for bt in range(nbt):
                for gb in range(HB):
                    ps = p2p.tile([128, nb], F32, tag="p2")
                    for hb in range(HB):
                        nc.tensor.matmul(
                            ps,
                            w2t[:, wi, hb, gb * 128 : (gb + 1) * 128],
                            h1[bt][:, hb, :],
                            start=(hb == 0),
                            stop=(hb == HB - 1),
                        )
                    bias = b2s[:, gb * E + e : gb * E + e + 1]
                    evac_relu(h2[bt][:, gb, :], ps, bias, bt * HB + gb + 1)

            # ---- layer 3: column-tiled accumulation into po, staggered.
            # Each pair is two concurrent matmuls in separate PE column
            # groups (tile_position (0,0) and (0,64)). ----
            def mk_l3(bt, g2, e=e, h2=h2, w3t=w3t, wi=wi):
                def go():
                    for gb in (2 * g2, 2 * g2 + 1):
                        half = (gb % 2) * DOUT
                        # two interleaved accumulation groups share the
                        # bank (column halves); per-element has_written
                        # bits keep this correct on HW
                        nc.tensor.matmul(
                            po[bt][half : half + DOUT, :],
                            w3t[:, wi, gb, :],
                            h2[bt][:, gb, :],
                            start=(e == 0 and gb < 2),
                            stop=(e == E - 1 and gb >= 2),
                            skip_group_check=True,
                        )

                return go

            if e == 0:
                po = [
                    pop.tile([128, nb], F32, tag="po", name=f"po{bt}")
                    for bt in range(nbt)
                ]
            pending = [mk_l3(bt, g2) for bt in range(nbt) for g2 in range(2)]

        for fn in pending:
            fn()

        # ---- epilogue: sum halves + bias, transpose to batch-major, store ----
        for bt in range(nbt):
            b0 = bt * nb
            ob = obp.tile([DOUT, nb], F32, tag="ob")
            # ob = (po_lo + b3sum) + po_hi — two ops; the engine may read only
            # one PSUM operand per instruction
            nc.vector.tensor_scalar_add(ob, po[bt][0:DOUT, :], b3sum)
            nc.vector.tensor_add(ob, ob, po[bt][DOUT : 2 * DOUT, :])
            pot = pop.tile([128, nt * DOUT], F32, tag="po", name=f"pot{bt}")
            for t in range(nt):
                nc.tensor.transpose(
                    pot[:, t * DOUT : (t + 1) * DOUT],
                    ob[:, t * 128 : (t + 1) * 128],
                    ident[:DOUT, :DOUT],
                )
            obt = obp.tile([128, nt * DOUT], F32, tag="obt")
            nc.vector.tensor_copy(obt, pot)
            nc.sync.dma_start(
                out=out[b0 : b0 + nb, :].rearrange("(t p) o -> p t o", p=128),
                in_=obt.rearrange("p (t o) -> p t o", o=DOUT),
            )

    nc.compile()
    return nc


_NC_CACHE = {}


def _get_nc():
    n_warm = int(os.environ.get("KERNEL_WARM", "12"))
    if n_warm not in _NC_CACHE:
        _NC_CACHE[n_warm] = build_nc(n_warm=n_warm)
    return _NC_CACHE[n_warm]


def kernel(x, W1, b1, W2, b2, W3, b3):
    x = np.ascontiguousarray(np.asarray(x, dtype=np.float32))
    ws = {
        "W1": np.ascontiguousarray(np.asarray(W1, dtype=np.float32)),
        "b1": np.ascontiguousarray(np.asarray(b1, dtype=np.float32)),
        "W2": np.ascontiguousarray(np.asarray(W2, dtype=np.float32)),
        "b2": np.ascontiguousarray(np.asarray(b2, dtype=np.float32)),
        "W3": np.ascontiguousarray(np.asarray(W3, dtype=np.float32)),
        "b3": np.ascontiguousarray(np.asarray(b3, dtype=np.float32)),
    }
    nc = _get_nc()
    shards = np.split(x, N_CORES, axis=0)
    in_maps = [{"x": np.ascontiguousarray(s), **ws} for s in shards]
    trace = bool(int(os.environ.get("KERNEL_TRACE", "0")))
    kwargs = {}
    if trace and os.environ.get("KERNEL_TRACE_DIR"):
        kwargs["tmpdir"] = os.environ["KERNEL_TRACE_DIR"]
    res = run_bass_kernel_spmd(nc, in_maps, list(range(N_CORES)), trace=trace, **kwargs)
    if trace:
        kernel.last_results = res
    return np.concatenate([res.results[c]["out"] for c in range(N_CORES)], axis=0)


# revision 27
# speedup vs baseline: 1.0556x; 1.0236x over previous
"""Trainium2 Bass kernel for grouped-expert 3-layer MLP (MoE, known covariance).

Computes, for x[B, E, DIN] and per-expert weights:
    h1 = relu(x[:,e] @ W1[e] + b1[e])      # [B, H]
    h2 = relu(h1 @ W2[e] + b2[e])          # [B, H]
    o  = h2 @ W3[e] + b3[e]                # [B, DOUT]
    out = sum_e o                          # [B, DOUT]

Sharding: data-parallel over batch across 8 NeuronCores (B=8192 -> 1024/core).
Weights are replicated to every core; no collectives needed.

Per-core schedule:
  - All matmuls run in bf16 (1 cycle/row on the PE at N=512; fp8 DoubleRow
    would be ~1.4x more but its quantization noise measures 3-6e-2 on this
    problem vs the 2e-2 gate; bf16 lands at ~4e-3). Weights and x are cast
    fp32->bf16 in flight by the gpsimd SWDGE DMA.
  - Expert-outer loop with double-buffered weights: the 23MB weight stream
    spreads over the whole kernel. (A block-outer variant that keeps all
    weights resident needs 2x the DMA rate and starves the PE; an XBAR
    DMA-transpose path for x double-hops through DRAM and loses to DMA
    queue serialization -- both measured slower.)
  - Weight loads are batched two experts per DMA instruction: the ~1us
    fixed SWDGE issue cost dominates per-instruction.
  - x tiles are PE-transposed to feature-major in bf16 (1 cyc/row + FWL
    weight load, ~113ns per 128x128 tile). The transposes for expert e+1
    and the column-tiled L3 pairs of expert e-1 are interleaved between
    L1(e)'s matmuls: L1 fills a full PSUM bank every 213ns if issued
    back-to-back, outrunning the ~700ns evacuations through the 3-slot
    p1 ring; the fillers pace it so the ring never blocks the PE.
  - Layer 3 (M=DOUT=64) is column-tiled: gb even -> PSUM partitions 0:64,
    gb odd -> 64:128 of the same bank, two concurrent matmuls in separate
    PE column groups, accumulated across all 16 experts; the halves are
    summed in the epilogue.
  - PSUM budget (8 banks): p1=3 (L1), p2=3 (L2 + transpose staging), po=2
    (per-block expert-sum accumulators).
  - PSUM evacuations (relu+bias, fp32->bf16) alternate between the ACT and
    DVE engines (GpSimd cannot read PSUM on TRN2).
  - A ~5us burst of identity matmuls at t=0 warms the PE HAM clock gate
    during the initial DMA wait (a cold PE runs at 1.2GHz, and the gate
    needs a full 3.4us activity window to open).
"""

import os
from contextlib import ExitStack

import bass_rust
import numpy as np

import concourse.bass as bass
import concourse.tile as tile
from concourse import bacc, mybir
from concourse.bass_utils import run_bass_kernel_spmd
from concourse.masks import make_identity

E, DIN, H, DOUT = 16, 128, 512, 64
B_FULL = 8192
N_CORES = 8
HB = H // 128  # 4 h-blocks
F32 = mybir.dt.float32
BF = mybir.dt.bfloat16


def build_nc(bloc=B_FULL // N_CORES, nb=512, n_warm=12):
    nbt = bloc // nb  # batch tiles per core
    nt = nb // 128
    assert bloc % nb == 0 and nb % 128 == 0

    nc = bacc.Bacc("TRN2", target_bir_lowering=False, debug=False)

    x = nc.dram_tensor("x", [bloc, E, DIN], F32, kind="ExternalInput")
    W1 = nc.dram_tensor("W1", [E, DIN, H], F32, kind="ExternalInput")
    b1 = nc.dram_tensor("b1", [E, H], F32, kind="ExternalInput")
    W2 = nc.dram_tensor("W2", [E, H, H], F32, kind="ExternalInput")
    b2 = nc.dram_tensor("b2", [E, H], F32, kind="ExternalInput")
    W3 = nc.dram_tensor("W3", [E, H, DOUT], F32, kind="ExternalInput")
    b3 = nc.dram_tensor("b3", [E, DOUT], F32, kind="ExternalInput")
    out = nc.dram_tensor("out", [bloc, DOUT], F32, kind="ExternalOutput")

    RELU = mybir.ActivationFunctionType.Relu
    ADD = mybir.AluOpType.add
    MAX = mybir.AluOpType.max

    with tile.TileContext(nc) as tc, ExitStack() as ctx:
        consts = ctx.enter_context(tc.tile_pool(name="consts", bufs=1))
        w1p = ctx.enter_context(tc.tile_pool(name="w1p", bufs=2))
        w2p = ctx.enter_context(tc.tile_pool(name="w2p", bufs=2))
        w3p = ctx.enter_context(tc.tile_pool(name="w3p", bufs=2))
        xp = ctx.enter_context(tc.tile_pool(name="xp", bufs=2))
        xtp = ctx.enter_context(tc.tile_pool(name="xtp", bufs=10))
        h1p = ctx.enter_context(tc.tile_pool(name="h1p", bufs=2))
        h2p = ctx.enter_context(tc.tile_pool(name="h2p", bufs=2))
        obp = ctx.enter_context(tc.tile_pool(name="obp", bufs=2))
        p1p = ctx.enter_context(tc.tile_pool(name="p1p", bufs=3, space="PSUM"))
        p2p = ctx.enter_context(tc.tile_pool(name="p2p", bufs=3, space="PSUM"))
        pop = ctx.enter_context(tc.tile_pool(name="pop", bufs=2, space="PSUM"))

        ident = consts.tile([128, 128], F32)
        make_identity(nc, ident)
        identr = consts.tile([128, 128], BF)
        nc.scalar.copy(identr, ident)

        # PE warmup: real (non-transpose) matmuls so the HAM clock gate sees
        # sustained activity and unthrottles 1.2 -> 2.4GHz before the first
        # data-dependent matmul issues.
        pjunk = p1p.tile([128, nb], F32, tag="p1", name="junk")
        jrhs = consts.tile([128, nb], BF)
        nc.vector.tensor_copy(jrhs[:, :128], identr)
        nc.vector.tensor_copy(jrhs[:, 128:256], identr)
        nc.vector.tensor_copy(jrhs[:, 256:384], identr)
        nc.vector.tensor_copy(jrhs[:, 384:], identr)
        for _ in range(n_warm):
            nc.tensor.matmul(pjunk, identr, jrhs, start=True, stop=True)

        # biases: load natural layout, PE-transpose so the per-feature bias
        # lands on partitions: b1s[p, hb*E + e] = b1[e, hb*128 + p]
        b1n = consts.tile([E, H], F32)
        nc.sync.dma_start(out=b1n, in_=b1[:, :])
        b2n = consts.tile([E, H], F32)
        nc.sync.dma_start(out=b2n, in_=b2[:, :])
        b3n = consts.tile([E, DOUT], F32)
        nc.sync.dma_start(out=b3n, in_=b3[:, :])
        b1s = consts.tile([128, HB * E], F32)
        b2s = consts.tile([128, HB * E], F32)
        for bn, bs in ((b1n, b1s), (b2n, b2s)):
            pb = p2p.tile([128, HB * E], F32, tag="p2", name="pb")
            for hb in range(HB):
                nc.tensor.transpose(
                    pb[:, hb * E : (hb + 1) * E],
                    bn[:, hb * 128 : (hb + 1) * 128],
                    ident[:E, :E],
                )
            nc.vector.tensor_copy(bs, pb)
        pb3 = p2p.tile([DOUT, E], F32, tag="p2", name="pb3")
        nc.tensor.transpose(pb3, b3n, ident[:E, :E])
        b3s = consts.tile([DOUT, E], F32)
        nc.vector.tensor_copy(b3s, pb3)
        b3sum = consts.tile([DOUT, 1], F32)
        nc.vector.reduce_sum(b3sum, b3s, axis=bass_rust.AxisListType.X)

        # relu+bias PSUM evacuation, rotated over 3 engines (ACT/DVE/GpSimd)
        # so consecutive PSUM-ring slots free up in parallel
        def evac_relu(out_ap, ps, bias, k):
            if k % 2 == 0:
                nc.scalar.activation(out_ap, ps, RELU, bias=bias)
            else:
                nc.vector.tensor_scalar(out_ap, ps, bias, 0.0, ADD, MAX)

        # ---- DMA emission helpers (gpsimd SWDGE, cast fp32->bf16) ----
        xin_tiles = {}

        def load_x(e):
            t = xp.tile([128, nbt, nt, DIN], BF, tag="xin", name="xin")
            nc.gpsimd.dma_start(
                out=t,
                in_=x[:, e, :].rearrange("(bt t p) d -> p bt t d", p=128, bt=nbt),
            )
            xin_tiles[e] = t

        w1_tiles = {}
        w23_tiles = {}

        def load_w1(e0, ne):
            es = slice(e0, e0 + ne)
            w1t = w1p.tile([DIN, ne, H], BF, tag="w1", name="w1")
            nc.gpsimd.dma_start(out=w1t, in_=W1[es].rearrange("e d h -> d e h"))
            for i in range(ne):
                w1_tiles[e0 + i] = (w1t, i)

        def load_w23(e0, ne):
            es = slice(e0, e0 + ne)
            w2t = w2p.tile([128, ne, HB, H], BF, tag="w2", name="w2")
            nc.gpsimd.dma_start(
                out=w2t, in_=W2[es].rearrange("e (hb p) g -> p e hb g", p=128)
            )
            w3t = w3p.tile([128, ne, HB, DOUT], BF, tag="w3", name="w3")
            nc.gpsimd.dma_start(
                out=w3t, in_=W3[es].rearrange("e (gb p) o -> p e gb o", p=128)
            )
            for i in range(ne):
                w23_tiles[e0 + i] = (w2t, w3t, i)

        def load_w(e0, ne):
            # one DMA instruction per weight tensor covering `ne` experts --
            # the ~1us fixed SWDGE issue cost dominates per-instruction
            load_w1(e0, ne)
            load_w23(e0, ne)

        # bootstrap loads: x first so the first transposes aren't stuck
        # behind the big weight transfers; single-expert weight loads up
        # front so L1/L2 of expert 0 start as early as possible
        load_x(0)
        load_w1(0, 1)
        load_x(1)
        load_w23(0, 1)
        load_w(1, 1)

        xt_tiles = {}

        def make_transpose_emitters(e):
            # PE transpose xin -> feature-major xt; one emitter per batch
            # tile so the transposes can be interleaved between L1 matmuls.
            # The PSUM staging tile comes from the p1 pool, claimed lazily at
            # first emitter call so it slots between the L1 allocations.
            xin = xin_tiles.pop(e)
            box = {}

            def mk(bt):
                def go():
                    if "pxt" not in box:
                        box["pxt"] = p2p.tile(
                            [DIN, nbt * nb], BF, tag="p2", name="pxt"
                        )
                    pxt = box["pxt"]
                    for t in range(nt):
                        nc.tensor.transpose(
                            pxt[:, bt * nb + t * 128 : bt * nb + (t + 1) * 128],
                            xin[:, bt, t, :],
                            identr,
                        )
                    xt = xtp.tile([DIN, nb], BF, tag="xt", name="xt")
                    if bt % 2 == 0:
                        nc.scalar.copy(xt, pxt[:, bt * nb : (bt + 1) * nb])
                    else:
                        nc.vector.tensor_copy(xt, pxt[:, bt * nb : (bt + 1) * nb])
                    xt_tiles[(e, bt)] = xt

                return go

            return [mk(bt) for bt in range(nbt)]

        for fn in make_transpose_emitters(0):
            fn()

        pending = []  # staggered L3 pair-emitters from the previous expert

        for e in range(E):
            if e + 2 < E:
                load_x(e + 2)
            if e % 2 == 0 and e + 2 < E:
                load_w(e + 2, 2)
            w1t, w1i = w1_tiles.pop(e)
            w2t, w3t, wi = w23_tiles.pop(e)

            # ---- layer 1 interleaved with prev-expert L3 pairs: L1 fills a
            # full PSUM bank every 213ns if issued back-to-back, outrunning
            # the ~700ns evacuations through the 3-slot ring; the L3 fillers
            # (which target the po banks) pace it so the ring never blocks
            # the PE. ----
            h1 = [
                h1p.tile([128, HB, nb], BF, tag=f"h1_{bt}", name=f"h1_{bt}")
                for bt in range(nbt)
            ]

            def emit_l1(bt, hb):
                ps = p1p.tile([128, nb], F32, tag="p1", name="ps1")
                nc.tensor.matmul(
                    ps,
                    w1t[:, w1i, hb * 128 : (hb + 1) * 128],
                    xt_l1[bt],
                    start=True,
                    stop=True,
                )
                bias = b1s[:, hb * E + e : hb * E + e + 1]
                evac_relu(h1[bt][:, hb, :], ps, bias, bt * HB + hb)

            xt_l1 = [xt_tiles.pop((e, bt)) for bt in range(nbt)]
            trs = make_transpose_emitters(e + 1) if e + 1 < E else []
            if e == 0:
                for bt in range(nbt):
                    for hb in range(HB):
                        emit_l1(bt, hb)
                for fn in trs:
                    fn()
            else:
                fillers = ([trs[0]] if trs else []) + pending[:2]
                fillers2 = ([trs[1]] if len(trs) > 1 else []) + pending[2:]
                emit_l1(0, 0)
                for hb in range(1, HB):
                    if fillers:
                        fillers.pop(0)()
                    emit_l1(0, hb)
                for hb in range(HB):
                    if fillers2:
                        fillers2.pop(0)()
                    emit_l1(1, hb)
                for fn in fillers + fillers2:
                    fn()
            pending = []

            # ---- layer 2, both batch tiles ----
            h2 = [h2p.tile([128, HB, nb], BF, tag=f"h2_{bt}", name=f"h2_{bt}") for bt in range(nbt)]
            for bt in range(nbt):
                for gb in range(HB):
                    ps = p2p.tile([128, nb], F32, tag="p2")
                    for hb in range(HB):
                        nc.tensor.matmul(
                            ps,
                            w2t[:, wi, hb, gb * 128 : (gb + 1) * 128],
                            h1[bt][:, hb, :],
                            start=(hb == 0),
                            stop=(hb == HB - 1),
                        )
                    bias = b2s[:, gb * E + e : gb * E + e + 1]
                    evac_relu(h2[bt][:, gb, :], ps, bias, bt * HB + gb + 1)

            # ---- layer 3: column-tiled accumulation into po, staggered.
            # Each pair is two concurrent matmuls in separate PE column
            # groups (tile_position (0,0) and (0,64)). ----
            def mk_l3(bt, g2, e=e, h2=h2, w3t=w3t, wi=wi):
                def go():
                    for gb in (2 * g2, 2 * g2 + 1):
                        half = (gb % 2) * DOUT
                        # two interleaved accumulation groups share the
                        # bank (column halves); per-element has_written
                        # bits keep this correct on HW
                        nc.tensor.matmul(
                            po[bt][half : half + DOUT, :],
                            w3t[:, wi, gb, :],
                            h2[bt][:, gb, :],
                            start=(e == 0 and gb < 2),
                            stop=(e == E - 1 and gb >= 2),
                            skip_group_check=True,
                        )

                return go

            if e == 0:
                po = [
                    pop.tile([128, nb], F32, tag="po", name=f"po{bt}")
                    for bt in range(nbt)
                ]
            pending = [mk_l3(bt, g2) for bt in range(nbt) for g2 in range(2)]

        # ---- final L3 + epilogue, overlapped: bt0's half-sum (DVE) runs
        # while bt1's last L3 pairs stream on the PE ----
        def ep_adds(bt):
            ob = obp.tile([DOUT, nb], F32, tag="ob", name=f"ob{bt}")
            # ob = (po_lo + b3sum) + po_hi — two ops; the engine may read only
            # one PSUM operand per instruction
            nc.vector.tensor_scalar_add(ob, po[bt][0:DOUT, :], b3sum)
            nc.vector.tensor_add(ob, ob, po[bt][DOUT : 2 * DOUT, :])
            return ob

        def ep_store(bt, ob):
            b0 = bt * nb
            pot = pop.tile([128, nt * DOUT], F32, tag="po", name=f"pot{bt}")
            for t in range(nt):
                nc.tensor.transpose(
                    pot[:, t * DOUT : (t + 1) * DOUT],
                    ob[:, t * 128 : (t + 1) * 128],
                    ident[:DOUT, :DOUT],
                )
            obt = obp.tile([128, nt * DOUT], F32, tag="obt", name=f"obt{bt}")
            nc.vector.tensor_copy(obt, pot)
            nc.sync.dma_start(
                out=out[b0 : b0 + nb, :].rearrange("(t p) o -> p t o", p=128),
                in_=obt.rearrange("p (t o) -> p t o", o=DOUT),
            )

        pending[0]()
        pending[1]()
        ob0 = ep_adds(0)
        pending[2]()
        pending[3]()
        ob1 = ep_adds(1)
        ep_store(0, ob0)
        ep_store(1, ob1)

    nc.compile()
    return nc


_NC_CACHE = {}


def _get_nc():
    n_warm = int(os.environ.get("KERNEL_WARM", "12"))
    if n_warm not in _NC_CACHE:
        _NC_CACHE[n_warm] = build_nc(n_warm=n_warm)
    return _NC_CACHE[n_warm]


def kernel(x, W1, b1, W2, b2, W3, b3):
    x = np.ascontiguousarray(np.asarray(x, dtype=np.float32))
    ws = {
        "W1": np.ascontiguousarray(np.asarray(W1, dtype=np.float32)),
        "b1": np.ascontiguousarray(np.asarray(b1, dtype=np.float32)),
        "W2": np.ascontiguousarray(np.asarray(W2, dtype=np.float32)),
        "b2": np.ascontiguousarray(np.asarray(b2, dtype=np.float32)),
        "W3": np.ascontiguousarray(np.asarray(W3, dtype=np.float32)),
        "b3": np.ascontiguousarray(np.asarray(b3, dtype=np.float32)),
    }
    nc = _get_nc()
    shards = np.split(x, N_CORES, axis=0)
    in_maps = [{"x": np.ascontiguousarray(s), **ws} for s in shards]
    trace = bool(int(os.environ.get("KERNEL_TRACE", "0")))
    kwargs = {}
    if trace and os.environ.get("KERNEL_TRACE_DIR"):
        kwargs["tmpdir"] = os.environ["KERNEL_TRACE_DIR"]
    res = run_bass_kernel_spmd(nc, in_maps, list(range(N_CORES)), trace=trace, **kwargs)
    if trace:
        kernel.last_results = res
    return np.concatenate([res.results[c]["out"] for c in range(N_CORES)], axis=0)


# revision 29
# speedup vs baseline: 1.0648x; 1.0087x over previous
"""Trainium2 Bass kernel for grouped-expert 3-layer MLP (MoE, known covariance).

Computes, for x[B, E, DIN] and per-expert weights:
    h1 = relu(x[:,e] @ W1[e] + b1[e])      # [B, H]
    h2 = relu(h1 @ W2[e] + b2[e])          # [B, H]
    o  = h2 @ W3[e] + b3[e]                # [B, DOUT]
    out = sum_e o                          # [B, DOUT]

Sharding: data-parallel over batch across 8 NeuronCores (B=8192 -> 1024/core).
Weights are replicated to every core; no collectives needed.

Per-core schedule:
  - All matmuls run in bf16 (1 cycle/row on the PE at N=512; fp8 DoubleRow
    would be ~1.4x more but its quantization noise measures 3-6e-2 on this
    problem vs the 2e-2 gate; bf16 lands at ~4e-3). Weights and x are cast
    fp32->bf16 in flight by the gpsimd SWDGE DMA.
  - Expert-outer loop with double-buffered weights: the 23MB weight stream
    spreads over the whole kernel. (A block-outer variant that keeps all
    weights resident needs 2x the DMA rate and starves the PE; an XBAR
    DMA-transpose path for x double-hops through DRAM and loses to DMA
    queue serialization -- both measured slower.)
  - Weight loads are batched two experts per DMA instruction: the ~1us
    fixed SWDGE issue cost dominates per-instruction.
  - x tiles are PE-transposed to feature-major in bf16 (1 cyc/row + FWL
    weight load, ~113ns per 128x128 tile). The transposes for expert e+1
    and the column-tiled L3 pairs of expert e-1 are interleaved between
    L1(e)'s matmuls: L1 fills a full PSUM bank every 213ns if issued
    back-to-back, outrunning the ~700ns evacuations through the 3-slot
    p1 ring; the fillers pace it so the ring never blocks the PE.
  - Layer 3 (M=DOUT=64) is column-tiled: gb even -> PSUM partitions 0:64,
    gb odd -> 64:128 of the same bank, two concurrent matmuls in separate
    PE column groups, accumulated across all 16 experts; the halves are
    summed in the epilogue.
  - PSUM budget (8 banks): p1=3 (L1), p2=3 (L2 + transpose staging), po=2
    (per-block expert-sum accumulators).
  - PSUM evacuations (relu+bias, fp32->bf16) alternate between the ACT and
    DVE engines (GpSimd cannot read PSUM on TRN2).
  - A ~5us burst of identity matmuls at t=0 warms the PE HAM clock gate
    during the initial DMA wait (a cold PE runs at 1.2GHz, and the gate
    needs a full 3.4us activity window to open).
"""

import os
from contextlib import ExitStack

import bass_rust
import numpy as np

import concourse.bass as bass
import concourse.tile as tile
from concourse import bacc, mybir
from concourse.bass_utils import run_bass_kernel_spmd
from concourse.masks import make_identity

E, DIN, H, DOUT = 16, 128, 512, 64
B_FULL = 8192
N_CORES = 8
HB = H // 128  # 4 h-blocks
F32 = mybir.dt.float32
BF = mybir.dt.bfloat16


def build_nc(bloc=B_FULL // N_CORES, nb=512, n_warm=12):
    nbt = bloc // nb  # batch tiles per core
    nt = nb // 128
    assert bloc % nb == 0 and nb % 128 == 0

    nc = bacc.Bacc("TRN2", target_bir_lowering=False, debug=False)

    x = nc.dram_tensor("x", [bloc, E, DIN], F32, kind="ExternalInput")
    W1 = nc.dram_tensor("W1", [E, DIN, H], F32, kind="ExternalInput")
    b1 = nc.dram_tensor("b1", [E, H], F32, kind="ExternalInput")
    W2 = nc.dram_tensor("W2", [E, H, H], F32, kind="ExternalInput")
    b2 = nc.dram_tensor("b2", [E, H], F32, kind="ExternalInput")
    W3 = nc.dram_tensor("W3", [E, H, DOUT], F32, kind="ExternalInput")
    b3 = nc.dram_tensor("b3", [E, DOUT], F32, kind="ExternalInput")
    out = nc.dram_tensor("out", [bloc, DOUT], F32, kind="ExternalOutput")

    RELU = mybir.ActivationFunctionType.Relu
    ADD = mybir.AluOpType.add
    MAX = mybir.AluOpType.max

    with tile.TileContext(nc) as tc, ExitStack() as ctx:
        consts = ctx.enter_context(tc.tile_pool(name="consts", bufs=1))
        w1p = ctx.enter_context(tc.tile_pool(name="w1p", bufs=2))
        w2p = ctx.enter_context(tc.tile_pool(name="w2p", bufs=2))
        w3p = ctx.enter_context(tc.tile_pool(name="w3p", bufs=2))
        xp = ctx.enter_context(tc.tile_pool(name="xp", bufs=2))
        xtp = ctx.enter_context(tc.tile_pool(name="xtp", bufs=10))
        h1p = ctx.enter_context(tc.tile_pool(name="h1p", bufs=2))
        h2p = ctx.enter_context(tc.tile_pool(name="h2p", bufs=2))
        obp = ctx.enter_context(tc.tile_pool(name="obp", bufs=2))
        p1p = ctx.enter_context(tc.tile_pool(name="p1p", bufs=3, space="PSUM"))
        p2p = ctx.enter_context(tc.tile_pool(name="p2p", bufs=3, space="PSUM"))
        pop = ctx.enter_context(tc.tile_pool(name="pop", bufs=2, space="PSUM"))

        ident = consts.tile([128, 128], F32)
        make_identity(nc, ident)
        identr = consts.tile([128, 128], BF)
        nc.scalar.copy(identr, ident)

        # PE warmup: real (non-transpose) matmuls so the HAM clock gate sees
        # sustained activity and unthrottles 1.2 -> 2.4GHz before the first
        # data-dependent matmul issues.
        pjunk = p1p.tile([128, nb], F32, tag="p1", name="junk")
        jrhs = consts.tile([128, nb], BF)
        nc.vector.tensor_copy(jrhs[:, :128], identr)
        nc.vector.tensor_copy(jrhs[:, 128:256], identr)
        nc.vector.tensor_copy(jrhs[:, 256:384], identr)
        nc.vector.tensor_copy(jrhs[:, 384:], identr)
        for _ in range(n_warm):
            nc.tensor.matmul(pjunk, identr, jrhs, start=True, stop=True)

        # biases: load natural layout, PE-transpose so the per-feature bias
        # lands on partitions: b1s[p, hb*E + e] = b1[e, hb*128 + p]
        b1n = consts.tile([E, H], F32)
        nc.sync.dma_start(out=b1n, in_=b1[:, :])
        b2n = consts.tile([E, H], F32)
        nc.sync.dma_start(out=b2n, in_=b2[:, :])
        b3n = consts.tile([E, DOUT], F32)
        nc.sync.dma_start(out=b3n, in_=b3[:, :])
        b1s = consts.tile([128, HB * E], F32)
        b2s = consts.tile([128, HB * E], F32)
        for bn, bs in ((b1n, b1s), (b2n, b2s)):
            pb = p2p.tile([128, HB * E], F32, tag="p2", name="pb")
            for hb in range(HB):
                nc.tensor.transpose(
                    pb[:, hb * E : (hb + 1) * E],
                    bn[:, hb * 128 : (hb + 1) * 128],
                    ident[:E, :E],
                )
            nc.vector.tensor_copy(bs, pb)
        pb3 = p2p.tile([DOUT, E], F32, tag="p2", name="pb3")
        nc.tensor.transpose(pb3, b3n, ident[:E, :E])
        b3s = consts.tile([DOUT, E], F32)
        nc.vector.tensor_copy(b3s, pb3)
        b3sum = consts.tile([DOUT, 1], F32)
        nc.vector.reduce_sum(b3sum, b3s, axis=bass_rust.AxisListType.X)

        # relu+bias PSUM evacuation, rotated over 3 engines (ACT/DVE/GpSimd)
        # so consecutive PSUM-ring slots free up in parallel
        def evac_relu(out_ap, ps, bias, k):
            if k % 2 == 0:
                nc.scalar.activation(out_ap, ps, RELU, bias=bias)
            else:
                nc.vector.tensor_scalar(out_ap, ps, bias, 0.0, ADD, MAX)

        # ---- DMA emission helpers (gpsimd SWDGE, cast fp32->bf16) ----
        xin_tiles = {}

        def load_x(e):
            t = xp.tile([128, nbt, nt, DIN], BF, tag="xin", name="xin")
            nc.gpsimd.dma_start(
                out=t,
                in_=x[:, e, :].rearrange("(bt t p) d -> p bt t d", p=128, bt=nbt),
            )
            xin_tiles[e] = t

        w1_tiles = {}
        w23_tiles = {}

        def load_w1(e0, ne):
            es = slice(e0, e0 + ne)
            w1t = w1p.tile([DIN, ne, H], BF, tag="w1", name="w1")
            nc.gpsimd.dma_start(out=w1t, in_=W1[es].rearrange("e d h -> d e h"))
            for i in range(ne):
                w1_tiles[e0 + i] = (w1t, i)

        def load_w23(e0, ne):
            es = slice(e0, e0 + ne)
            w2t = w2p.tile([128, ne, HB, H], BF, tag="w2", name="w2")
            nc.gpsimd.dma_start(
                out=w2t, in_=W2[es].rearrange("e (hb p) g -> p e hb g", p=128)
            )
            w3t = w3p.tile([128, ne, HB, DOUT], BF, tag="w3", name="w3")
            nc.gpsimd.dma_start(
                out=w3t, in_=W3[es].rearrange("e (gb p) o -> p e gb o", p=128)
            )
            for i in range(ne):
                w23_tiles[e0 + i] = (w2t, w3t, i)

        def load_w(e0, ne):
            # one DMA instruction per weight tensor covering `ne` experts --
            # the ~1us fixed SWDGE issue cost dominates per-instruction
            load_w1(e0, ne)
            load_w23(e0, ne)

        # bootstrap loads: x first so the first transposes aren't stuck
        # behind the big weight transfers; single-expert weight loads up
        # front so L1/L2 of expert 0 start as early as possible
        load_x(0)
        load_w1(0, 1)
        load_x(1)
        load_w23(0, 1)
        load_w(1, 1)

        xt_tiles = {}

        def make_transpose_emitters(e):
            # PE transpose xin -> feature-major xt; one emitter per batch
            # tile so the transposes can be interleaved between L1 matmuls.
            # The PSUM staging tile comes from the p1 pool, claimed lazily at
            # first emitter call so it slots between the L1 allocations.
            xin = xin_tiles.pop(e)
            box = {}

            def mk(bt):
                def go():
                    if "pxt" not in box:
                        box["pxt"] = p2p.tile(
                            [DIN, nbt * nb], BF, tag="p2", name="pxt"
                        )
                    pxt = box["pxt"]
                    for t in range(nt):
                        nc.tensor.transpose(
                            pxt[:, bt * nb + t * 128 : bt * nb + (t + 1) * 128],
                            xin[:, bt, t, :],
                            identr,
                        )

                def copy_out(bt=bt):
                    # the PSUM->SBUF copy is deferred to the L2 window so it
                    # doesn't delay the L1 evacuations on ACT/DVE
                    xt = xtp.tile([DIN, nb], BF, tag="xt", name="xt")
                    if bt % 2 == 0:
                        nc.scalar.copy(xt, box["pxt"][:, bt * nb : (bt + 1) * nb])
                    else:
                        nc.vector.tensor_copy(
                            xt, box["pxt"][:, bt * nb : (bt + 1) * nb]
                        )
                    xt_tiles[(e, bt)] = xt

                return go, copy_out

            return [mk(bt) for bt in range(nbt)]

        for fn, cp in make_transpose_emitters(0):
            fn()
            cp()

        pending = []  # staggered L3 pair-emitters from the previous expert

        for e in range(E):
            if e + 2 < E:
                load_x(e + 2)
            if e % 2 == 0 and e + 2 < E:
                load_w(e + 2, 2)
            w1t, w1i = w1_tiles.pop(e)
            w2t, w3t, wi = w23_tiles.pop(e)

            # ---- layer 1 interleaved with prev-expert L3 pairs: L1 fills a
            # full PSUM bank every 213ns if issued back-to-back, outrunning
            # the ~700ns evacuations through the 3-slot ring; the L3 fillers
            # (which target the po banks) pace it so the ring never blocks
            # the PE. ----
            h1 = [
                h1p.tile([128, HB, nb], BF, tag=f"h1_{bt}", name=f"h1_{bt}")
                for bt in range(nbt)
            ]

            def emit_l1(bt, hb):
                ps = p1p.tile([128, nb], F32, tag="p1", name="ps1")
                nc.tensor.matmul(
                    ps,
                    w1t[:, w1i, hb * 128 : (hb + 1) * 128],
                    xt_l1[bt],
                    start=True,
                    stop=True,
                )
                bias = b1s[:, hb * E + e : hb * E + e + 1]
                evac_relu(h1[bt][:, hb, :], ps, bias, bt * HB + hb)

            xt_l1 = [xt_tiles.pop((e, bt)) for bt in range(nbt)]
            trs = make_transpose_emitters(e + 1) if e + 1 < E else []
            if e == 0:
                for bt in range(nbt):
                    for hb in range(HB):
                        emit_l1(bt, hb)
                for fn, cp in trs:
                    fn()
                    cp()
                trs = []
            else:
                fillers = ([trs[0][0]] if trs else []) + pending[:2]
                fillers2 = ([trs[1][0]] if len(trs) > 1 else []) + pending[2:]
                emit_l1(0, 0)
                for hb in range(1, HB):
                    if fillers:
                        fillers.pop(0)()
                    emit_l1(0, hb)
                for hb in range(HB):
                    if fillers2:
                        fillers2.pop(0)()
                    emit_l1(1, hb)
                for fn in fillers + fillers2:
                    fn()
            pending = []

            # ---- layer 2, both batch tiles ----
            for _, cp in trs:
                cp()
            h2 = [h2p.tile([128, HB, nb], BF, tag=f"h2_{bt}", name=f"h2_{bt}") for bt in range(nbt)]
            for bt in range(nbt):
                for gb in range(HB):
                    ps = p2p.tile([128, nb], F32, tag="p2")
                    for hb in range(HB):
                        nc.tensor.matmul(
                            ps,
                            w2t[:, wi, hb, gb * 128 : (gb + 1) * 128],
                            h1[bt][:, hb, :],
                            start=(hb == 0),
                            stop=(hb == HB - 1),
                        )
                    bias = b2s[:, gb * E + e : gb * E + e + 1]
                    evac_relu(h2[bt][:, gb, :], ps, bias, bt * HB + gb + 1)

            # ---- layer 3: column-tiled accumulation into po, staggered.
            # Each pair is two concurrent matmuls in separate PE column
            # groups (tile_position (0,0) and (0,64)). ----
            def mk_l3(bt, g2, e=e, h2=h2, w3t=w3t, wi=wi):
                def go():
                    for gb in (2 * g2, 2 * g2 + 1):
                        half = (gb % 2) * DOUT
                        # two interleaved accumulation groups share the
                        # bank (column halves); per-element has_written
                        # bits keep this correct on HW
                        nc.tensor.matmul(
                            po[bt][half : half + DOUT, :],
                            w3t[:, wi, gb, :],
                            h2[bt][:, gb, :],
                            start=(e == 0 and gb < 2),
                            stop=(e == E - 1 and gb >= 2),
                            skip_group_check=True,
                        )

                return go

            if e == 0:
                po = [
                    pop.tile([128, nb], F32, tag="po", name=f"po{bt}")
                    for bt in range(nbt)
                ]
            pending = [mk_l3(bt, g2) for bt in range(nbt) for g2 in range(2)]

        # ---- final L3 + epilogue, overlapped: bt0's half-sum (DVE) runs
        # while bt1's last L3 pairs stream on the PE ----
        def ep_adds(bt):
            ob = obp.tile([DOUT, nb], F32, tag="ob", name=f"ob{bt}")
            # ob = (po_lo + b3sum) + po_hi — two ops; the engine may read only
            # one PSUM operand per instruction
            nc.scalar.activation(
                ob, po[bt][0:DOUT, :], mybir.ActivationFunctionType.Identity,
                bias=b3sum,
            )
            nc.vector.tensor_add(ob, ob, po[bt][DOUT : 2 * DOUT, :])
            return ob

        def ep_store(bt, ob):
            b0 = bt * nb
            pot = pop.tile([128, nt * DOUT], F32, tag="po", name=f"pot{bt}")
            for t in range(nt):
                nc.tensor.transpose(
                    pot[:, t * DOUT : (t + 1) * DOUT],
                    ob[:, t * 128 : (t + 1) * 128],
                    ident[:DOUT, :DOUT],
                )
            obt = obp.tile([128, nt * DOUT], F32, tag="obt", name=f"obt{bt}")
            nc.vector.tensor_copy(obt, pot)
            nc.sync.dma_start(
                out=out[b0 : b0 + nb, :].rearrange("(t p) o -> p t o", p=128),
                in_=obt.rearrange("p (t o) -> p t o", o=DOUT),
            )

        pending[0]()
        pending[1]()
        ob0 = ep_adds(0)
        pending[2]()
        pending[3]()
        ob1 = ep_adds(1)
        ep_store(0, ob0)
        ep_store(1, ob1)

    nc.compile()
    return nc


_NC_CACHE = {}


def _get_nc():
    n_warm = int(os.environ.get("KERNEL_WARM", "12"))
    if n_warm not in _NC_CACHE:
        _NC_CACHE[n_warm] = build_nc(n_warm=n_warm)
    return _NC_CACHE[n_warm]


def kernel(x, W1, b1, W2, b2, W3, b3):
    x = np.ascontiguousarray(np.asarray(x, dtype=np.float32))
    ws = {
        "W1": np.ascontiguousarray(np.asarray(W1, dtype=np.float32)),
        "b1": np.ascontiguousarray(np.asarray(b1, dtype=np.float32)),
        "W2": np.ascontiguousarray(np.asarray(W2, dtype=np.float32)),
        "b2": np.ascontiguousarray(np.asarray(b2, dtype=np.float32)),
        "W3": np.ascontiguousarray(np.asarray(W3, dtype=np.float32)),
        "b3": np.ascontiguousarray(np.asarray(b3, dtype=np.float32)),
    }
    nc = _get_nc()
    shards = np.split(x, N_CORES, axis=0)
    in_maps = [{"x": np.ascontiguousarray(s), **ws} for s in shards]
    trace = bool(int(os.environ.get("KERNEL_TRACE", "0")))
    kwargs = {}
    if trace and os.environ.get("KERNEL_TRACE_DIR"):
        kwargs["tmpdir"] = os.environ["KERNEL_TRACE_DIR"]
    res = run_bass_kernel_spmd(nc, in_maps, list(range(N_CORES)), trace=trace, **kwargs)
    if trace:
        kernel.last_results = res
    return np.concatenate([res.results[c]["out"] for c in range(N_CORES)], axis=0)


# revision 31
# speedup vs baseline: 1.0655x; 1.0006x over previous
"""Trainium2 Bass kernel for grouped-expert 3-layer MLP (MoE, known covariance).

Computes, for x[B, E, DIN] and per-expert weights:
    h1 = relu(x[:,e] @ W1[e] + b1[e])      # [B, H]
    h2 = relu(h1 @ W2[e] + b2[e])          # [B, H]
    o  = h2 @ W3[e] + b3[e]                # [B, DOUT]
    out = sum_e o                          # [B, DOUT]

Sharding: data-parallel over batch across 8 NeuronCores (B=8192 -> 1024/core).
Weights are replicated to every core; no collectives needed.

Per-core schedule:
  - All matmuls run in bf16 (1 cycle/row on the PE at N=512; fp8 DoubleRow
    would be ~1.4x more but its quantization noise measures 3-6e-2 on this
    problem vs the 2e-2 gate; bf16 lands at ~4e-3). Weights and x are cast
    fp32->bf16 in flight by the gpsimd SWDGE DMA.
  - Expert-outer loop with double-buffered weights: the 23MB weight stream
    spreads over the whole kernel. (A block-outer variant that keeps all
    weights resident needs 2x the DMA rate and starves the PE; an XBAR
    DMA-transpose path for x double-hops through DRAM and loses to DMA
    queue serialization -- both measured slower.)
  - Weight loads are batched two experts per DMA instruction: the ~1us
    fixed SWDGE issue cost dominates per-instruction.
  - x tiles are PE-transposed to feature-major in bf16 (1 cyc/row + FWL
    weight load, ~113ns per 128x128 tile). The transposes for expert e+1
    and the column-tiled L3 pairs of expert e-1 are interleaved between
    L1(e)'s matmuls: L1 fills a full PSUM bank every 213ns if issued
    back-to-back, outrunning the ~700ns evacuations through the 3-slot
    p1 ring; the fillers pace it so the ring never blocks the PE.
  - Layer 3 (M=DOUT=64) is column-tiled: gb even -> PSUM partitions 0:64,
    gb odd -> 64:128 of the same bank, two concurrent matmuls in separate
    PE column groups, accumulated across all 16 experts; the halves are
    summed in the epilogue.
  - PSUM budget (8 banks): p1=3 (L1), p2=3 (L2 + transpose staging), po=2
    (per-block expert-sum accumulators).
  - PSUM evacuations (relu+bias, fp32->bf16) alternate between the ACT and
    DVE engines (GpSimd cannot read PSUM on TRN2).
  - A ~5us burst of identity matmuls at t=0 warms the PE HAM clock gate
    during the initial DMA wait (a cold PE runs at 1.2GHz, and the gate
    needs a full 3.4us activity window to open).
"""

import os
from contextlib import ExitStack

import bass_rust
import numpy as np

import concourse.bass as bass
import concourse.tile as tile
from concourse import bacc, mybir
from concourse.bass_utils import run_bass_kernel_spmd
from concourse.masks import make_identity

E, DIN, H, DOUT = 16, 128, 512, 64
B_FULL = 8192
N_CORES = 8
HB = H // 128  # 4 h-blocks
F32 = mybir.dt.float32
BF = mybir.dt.bfloat16


def build_nc(bloc=B_FULL // N_CORES, nb=512, n_warm=12):
    nbt = bloc // nb  # batch tiles per core
    nt = nb // 128
    assert bloc % nb == 0 and nb % 128 == 0

    nc = bacc.Bacc("TRN2", target_bir_lowering=False, debug=False)

    x = nc.dram_tensor("x", [bloc, E, DIN], F32, kind="ExternalInput")
    W1 = nc.dram_tensor("W1", [E, DIN, H], F32, kind="ExternalInput")
    b1 = nc.dram_tensor("b1", [E, H], F32, kind="ExternalInput")
    W2 = nc.dram_tensor("W2", [E, H, H], F32, kind="ExternalInput")
    b2 = nc.dram_tensor("b2", [E, H], F32, kind="ExternalInput")
    W3 = nc.dram_tensor("W3", [E, H, DOUT], F32, kind="ExternalInput")
    b3 = nc.dram_tensor("b3", [E, DOUT], F32, kind="ExternalInput")
    out = nc.dram_tensor("out", [bloc, DOUT], F32, kind="ExternalOutput")

    RELU = mybir.ActivationFunctionType.Relu
    ADD = mybir.AluOpType.add
    MAX = mybir.AluOpType.max

    with tile.TileContext(nc) as tc, ExitStack() as ctx:
        consts = ctx.enter_context(tc.tile_pool(name="consts", bufs=1))
        w1p = ctx.enter_context(tc.tile_pool(name="w1p", bufs=2))
        w2p = ctx.enter_context(tc.tile_pool(name="w2p", bufs=2))
        w3p = ctx.enter_context(tc.tile_pool(name="w3p", bufs=2))
        xp = ctx.enter_context(tc.tile_pool(name="xp", bufs=2))
        xtp = ctx.enter_context(tc.tile_pool(name="xtp", bufs=10))
        h1p = ctx.enter_context(tc.tile_pool(name="h1p", bufs=2))
        h2p = ctx.enter_context(tc.tile_pool(name="h2p", bufs=2))
        obp = ctx.enter_context(tc.tile_pool(name="obp", bufs=2))
        p1p = ctx.enter_context(tc.tile_pool(name="p1p", bufs=3, space="PSUM"))
        p2p = ctx.enter_context(tc.tile_pool(name="p2p", bufs=3, space="PSUM"))
        pop = ctx.enter_context(tc.tile_pool(name="pop", bufs=2, space="PSUM"))

        ident = consts.tile([128, 128], F32)
        make_identity(nc, ident)
        identr = consts.tile([128, 128], BF)
        nc.scalar.copy(identr, ident)

        # PE warmup: real (non-transpose) matmuls so the HAM clock gate sees
        # sustained activity and unthrottles 1.2 -> 2.4GHz before the first
        # data-dependent matmul issues.
        pjunk = p1p.tile([128, nb], F32, tag="p1", name="junk")
        jrhs = consts.tile([128, nb], BF)
        nc.vector.tensor_copy(jrhs[:, :128], identr)
        nc.vector.tensor_copy(jrhs[:, 128:256], identr)
        nc.vector.tensor_copy(jrhs[:, 256:384], identr)
        nc.vector.tensor_copy(jrhs[:, 384:], identr)
        for _ in range(n_warm):
            nc.tensor.matmul(pjunk, identr, jrhs, start=True, stop=True)

        # biases: load natural layout, PE-transpose so the per-feature bias
        # lands on partitions: b1s[p, hb*E + e] = b1[e, hb*128 + p]
        b1n = consts.tile([E, H], F32)
        nc.sync.dma_start(out=b1n, in_=b1[:, :])
        b2n = consts.tile([E, H], F32)
        nc.sync.dma_start(out=b2n, in_=b2[:, :])
        b3n = consts.tile([E, DOUT], F32)
        nc.sync.dma_start(out=b3n, in_=b3[:, :])
        b1s = consts.tile([128, HB * E], F32)
        b2s = consts.tile([128, HB * E], F32)
        for bn, bs in ((b1n, b1s), (b2n, b2s)):
            pb = p2p.tile([128, HB * E], F32, tag="p2", name="pb")
            for hb in range(HB):
                nc.tensor.transpose(
                    pb[:, hb * E : (hb + 1) * E],
                    bn[:, hb * 128 : (hb + 1) * 128],
                    ident[:E, :E],
                )
            nc.vector.tensor_copy(bs, pb)
        pb3 = p2p.tile([DOUT, E], F32, tag="p2", name="pb3")
        nc.tensor.transpose(pb3, b3n, ident[:E, :E])
        b3s = consts.tile([DOUT, E], F32)
        nc.vector.tensor_copy(b3s, pb3)
        b3sum = consts.tile([DOUT, 1], F32)
        nc.vector.reduce_sum(b3sum, b3s, axis=bass_rust.AxisListType.X)

        # relu+bias PSUM evacuation, rotated over 3 engines (ACT/DVE/GpSimd)
        # so consecutive PSUM-ring slots free up in parallel
        def evac_relu(out_ap, ps, bias, k):
            if k % 2 == 0:
                nc.scalar.activation(out_ap, ps, RELU, bias=bias)
            else:
                nc.vector.tensor_scalar(out_ap, ps, bias, 0.0, ADD, MAX)

        # ---- DMA emission helpers (gpsimd SWDGE, cast fp32->bf16) ----
        xin_tiles = {}

        def load_x(e):
            t = xp.tile([128, nbt, nt, DIN], BF, tag="xin", name="xin")
            nc.gpsimd.dma_start(
                out=t,
                in_=x[:, e, :].rearrange("(bt t p) d -> p bt t d", p=128, bt=nbt),
            )
            xin_tiles[e] = t

        w1_tiles = {}
        w23_tiles = {}

        def load_w1(e0, ne):
            es = slice(e0, e0 + ne)
            w1t = w1p.tile([DIN, ne, H], BF, tag="w1", name="w1")
            nc.gpsimd.dma_start(out=w1t, in_=W1[es].rearrange("e d h -> d e h"))
            for i in range(ne):
                w1_tiles[e0 + i] = (w1t, i)

        def load_w23(e0, ne):
            es = slice(e0, e0 + ne)
            w2t = w2p.tile([128, ne, HB, H], BF, tag="w2", name="w2")
            nc.gpsimd.dma_start(
                out=w2t, in_=W2[es].rearrange("e (hb p) g -> p e hb g", p=128)
            )
            w3t = w3p.tile([128, ne, HB, DOUT], BF, tag="w3", name="w3")
            nc.gpsimd.dma_start(
                out=w3t, in_=W3[es].rearrange("e (gb p) o -> p e gb o", p=128)
            )
            for i in range(ne):
                w23_tiles[e0 + i] = (w2t, w3t, i)

        def load_w(e0, ne):
            # one DMA instruction per weight tensor covering `ne` experts --
            # the ~1us fixed SWDGE issue cost dominates per-instruction
            load_w1(e0, ne)
            load_w23(e0, ne)

        # bootstrap loads: x first so the first transposes aren't stuck
        # behind the big weight transfers; single-expert weight loads up
        # front so L1/L2 of expert 0 start as early as possible
        load_x(0)
        load_w1(0, 1)
        load_x(1)
        load_w23(0, 1)
        load_w(1, 1)

        xt_tiles = {}

        def make_transpose_emitters(e):
            # PE transpose xin -> feature-major xt; one emitter per batch
            # tile so the transposes can be interleaved between L1 matmuls.
            # The PSUM staging tile comes from the p1 pool, claimed lazily at
            # first emitter call so it slots between the L1 allocations.
            xin = xin_tiles.pop(e)
            box = {}

            def mk(bt):
                def go():
                    if "pxt" not in box:
                        box["pxt"] = p2p.tile(
                            [DIN, nbt * nb], BF, tag="p2", name="pxt"
                        )
                    pxt = box["pxt"]
                    for t in range(nt):
                        nc.tensor.transpose(
                            pxt[:, bt * nb + t * 128 : bt * nb + (t + 1) * 128],
                            xin[:, bt, t, :],
                            identr,
                        )

                def copy_out(bt=bt):
                    # the PSUM->SBUF copy is deferred to the L2 window so it
                    # doesn't delay the L1 evacuations on ACT/DVE
                    xt = xtp.tile([DIN, nb], BF, tag="xt", name="xt")
                    if bt % 2 == 0:
                        nc.scalar.copy(xt, box["pxt"][:, bt * nb : (bt + 1) * nb])
                    else:
                        nc.vector.tensor_copy(
                            xt, box["pxt"][:, bt * nb : (bt + 1) * nb]
                        )
                    xt_tiles[(e, bt)] = xt

                return go, copy_out

            return [mk(bt) for bt in range(nbt)]

        for fn, cp in make_transpose_emitters(0):
            fn()
            cp()

        pending = []  # staggered L3 pair-emitters from the previous expert

        for e in range(E):
            if e + 2 < E:
                load_x(e + 2)
            if e % 2 == 0 and e + 2 < E:
                load_w(e + 2, 2)
            w1t, w1i = w1_tiles.pop(e)
            w2t, w3t, wi = w23_tiles.pop(e)

            # ---- layer 1 interleaved with prev-expert L3 pairs: L1 fills a
            # full PSUM bank every 213ns if issued back-to-back, outrunning
            # the ~700ns evacuations through the 3-slot ring; the L3 fillers
            # (which target the po banks) pace it so the ring never blocks
            # the PE. ----
            h1 = [
                h1p.tile([128, HB, nb], BF, tag=f"h1_{bt}", name=f"h1_{bt}")
                for bt in range(nbt)
            ]

            def emit_l1(bt, hb):
                ps = p1p.tile([128, nb], F32, tag="p1", name="ps1")
                nc.tensor.matmul(
                    ps,
                    w1t[:, w1i, hb * 128 : (hb + 1) * 128],
                    xt_l1[bt],
                    start=True,
                    stop=True,
                )
                bias = b1s[:, hb * E + e : hb * E + e + 1]
                evac_relu(h1[bt][:, hb, :], ps, bias, bt * HB + hb)

            xt_l1 = [xt_tiles.pop((e, bt)) for bt in range(nbt)]
            trs = make_transpose_emitters(e + 1) if e + 1 < E else []
            if e == 0:
                for bt in range(nbt):
                    for hb in range(HB):
                        emit_l1(bt, hb)
                for fn, cp in trs:
                    fn()
                    cp()
                trs = []
            else:
                fillers = ([trs[0][0]] if trs else []) + pending[:2]
                fillers2 = ([trs[1][0]] if len(trs) > 1 else []) + pending[2:]
                emit_l1(0, 0)
                for hb in range(1, HB):
                    if fillers:
                        fillers.pop(0)()
                    emit_l1(0, hb)
                for hb in range(HB):
                    if fillers2:
                        fillers2.pop(0)()
                    emit_l1(1, hb)
                for fn in fillers + fillers2:
                    fn()
            pending = []

            # ---- layer 2, both batch tiles ----
            for _, cp in trs:
                cp()
            h2 = [h2p.tile([128, HB, nb], BF, tag=f"h2_{bt}", name=f"h2_{bt}") for bt in range(nbt)]
            for bt in range(nbt):
                for gb in range(HB):
                    ps = p2p.tile([128, nb], F32, tag="p2")
                    for hb in range(HB):
                        nc.tensor.matmul(
                            ps,
                            w2t[:, wi, hb, gb * 128 : (gb + 1) * 128],
                            h1[bt][:, hb, :],
                            start=(hb == 0),
                            stop=(hb == HB - 1),
                        )
                    bias = b2s[:, gb * E + e : gb * E + e + 1]
                    evac_relu(h2[bt][:, gb, :], ps, bias, bt * HB + gb + 1)

            # ---- layer 3: column-tiled accumulation into po, staggered.
            # Each pair is two concurrent matmuls in separate PE column
            # groups (tile_position (0,0) and (0,64)). ----
            def mk_l3(bt, g2, e=e, h2=h2, w3t=w3t, wi=wi):
                def go():
                    for gb in (2 * g2, 2 * g2 + 1):
                        half = (gb % 2) * DOUT
                        # two interleaved accumulation groups share the
                        # bank (column halves); per-element has_written
                        # bits keep this correct on HW
                        nc.tensor.matmul(
                            po[bt][half : half + DOUT, :],
                            w3t[:, wi, gb, :],
                            h2[bt][:, gb, :],
                            start=(e == 0 and gb < 2),
                            stop=(e == E - 1 and gb >= 2),
                            skip_group_check=True,
                        )

                return go

            if e == 0:
                po = [
                    pop.tile([128, nb], F32, tag="po", name=f"po{bt}")
                    for bt in range(nbt)
                ]
            pending = [mk_l3(bt, g2) for bt in range(nbt) for g2 in range(2)]

        # ---- final L3 + epilogue, overlapped: bt0's half-sum (DVE) runs
        # while bt1's last L3 pairs stream on the PE ----
        def ep_adds(bt):
            ob = obp.tile([DOUT, nb], F32, tag="ob", name=f"ob{bt}")
            # ob = (po_lo + b3sum) + po_hi — two ops; the engine may read only
            # one PSUM operand per instruction
            nc.scalar.activation(
                ob, po[bt][0:DOUT, :], mybir.ActivationFunctionType.Identity,
                bias=b3sum,
            )
            nc.vector.tensor_add(ob, ob, po[bt][DOUT : 2 * DOUT, :])
            return ob

        def ep_store(bt, ob):
            b0 = bt * nb
            pot = pop.tile([128, nt * DOUT], F32, tag="po", name=f"pot{bt}")
            for t in range(nt):
                nc.tensor.transpose(
                    pot[:, t * DOUT : (t + 1) * DOUT],
                    ob[:, t * 128 : (t + 1) * 128],
                    ident[:DOUT, :DOUT],
                )
            obt = obp.tile([128, nt * DOUT], F32, tag="obt", name=f"obt{bt}")
            nc.vector.tensor_copy(obt, pot)
            nc.sync.dma_start(
                out=out[b0 : b0 + nb, :].rearrange("(t p) o -> p t o", p=128),
                in_=obt.rearrange("p (t o) -> p t o", o=DOUT),
            )

        pending[0]()
        pending[1]()
        ob0 = ep_adds(0)
        pending[2]()
        pending[3]()
        ob1 = ep_adds(1)
        ep_store(0, ob0)
        ep_store(1, ob1)

    nc.compile()
    return nc


_NC_CACHE = {}


def _get_nc():
    n_warm = int(os.environ.get("KERNEL_WARM", "12"))
    if n_warm not in _NC_CACHE:
        _NC_CACHE[n_warm] = build_nc(n_warm=n_warm)
    return _NC_CACHE[n_warm]


def kernel(x, W1, b1, W2, b2, W3, b3):
    x = np.ascontiguousarray(np.asarray(x, dtype=np.float32))
    ws = {
        "W1": np.ascontiguousarray(np.asarray(W1, dtype=np.float32)),
        "b1": np.ascontiguousarray(np.asarray(b1, dtype=np.float32)),
        "W2": np.ascontiguousarray(np.asarray(W2, dtype=np.float32)),
        "b2": np.ascontiguousarray(np.asarray(b2, dtype=np.float32)),
        "W3": np.ascontiguousarray(np.asarray(W3, dtype=np.float32)),
        "b3": np.ascontiguousarray(np.asarray(b3, dtype=np.float32)),
    }
    nc = _get_nc()
    shards = np.split(x, N_CORES, axis=0)
    in_maps = [{"x": np.ascontiguousarray(s), **ws} for s in shards]
    trace = bool(int(os.environ.get("KERNEL_TRACE", "0")))
    kwargs = {}
    if trace and os.environ.get("KERNEL_TRACE_DIR"):
        kwargs["tmpdir"] = os.environ["KERNEL_TRACE_DIR"]
    res = run_bass_kernel_spmd(nc, in_maps, list(range(N_CORES)), trace=trace, **kwargs)
    if trace:
        kernel.last_results = res
    return np.concatenate([res.results[c]["out"] for c in range(N_CORES)], axis=0)
